# revision 9
# baseline (speedup 1.0000x reference)
"""Trainium2 Bass kernel for nn_CLIPVisionTower_Nuwa_abli (topk_masking).

Per-image pipeline (pure batch data-parallel, 1 image per NeuronCore):
  metric  = sum over heads of attn[:, 0, 1:]                  [576]
  mask out each 2x2-region argmin, take global top-42 of the
  remainder, bench = those patch indices ascending             [42]
  nrm     = l2-normalized mean over 9 layers of hs[:, 1:, :]   [576,1024]
  sim     = nrm[bench] @ nrm.T                                 [42,576]
  w       = relu(sim) * dist_penalty[bench]; row-normalize; self weight 1
  out     = w @ hidden_agg[1:, :]                              [42,1024]
"""

import math

import numpy as np

B = 8
HGRID = 24
P = 576  # patches
K = 42  # bench tokens
HEADS = 16
D = 1024
L = 9  # layers
NCORES = 8
PT = [(0, 128), (128, 128), (256, 128), (384, 128), (512, 64)]  # patch tiles
ND = D // 128  # 8 d-chunks of 128
NEG_BIG = -1.0e30
KEY_BIG = 1.0e9


def _dist_penalty_np() -> np.ndarray:
    ys, xs = np.meshgrid(
        np.arange(HGRID, dtype=np.float32),
        np.arange(HGRID, dtype=np.float32),
        indexing="ij",
    )
    coords = np.stack([ys, xs], axis=-1).reshape(-1, 2)
    diff = coords[:, None, :] - coords[None, :, :]
    dist = np.sqrt((diff * diff).sum(axis=-1), dtype=np.float32)
    thresh = np.float32(math.sqrt(280.0))
    return (np.float32(1.0) - np.minimum(dist / thresh, np.float32(1.0))).astype(
        np.float32
    )


def _emit(nc, tc, ctx):
    import concourse.mybir as mybir
    from concourse.masks import make_identity

    f32 = mybir.dt.float32
    i32 = mybir.dt.int32
    i16 = mybir.dt.int16
    Alu = mybir.AluOpType
    Act = mybir.ActivationFunctionType
    AX = mybir.AxisListType

    attn = nc.dram_tensor("attn", [HEADS, P + 1, P + 1], f32, kind="ExternalInput").ap()
    hagg = nc.dram_tensor("hidden_agg", [P + 1, D], f32, kind="ExternalInput").ap()
    hs = nc.dram_tensor("stacked_hs", [L, P + 1, D], f32, kind="ExternalInput").ap()
    dist = nc.dram_tensor("dist", [P, P], f32, kind="ExternalInput").ap()
    out_d = nc.dram_tensor("out", [K, D], f32, kind="ExternalOutput").ap()
    bench_d = nc.dram_tensor("bench", [1, K], i32, kind="ExternalOutput").ap()

    consts = ctx.enter_context(tc.tile_pool(name="consts", bufs=1))
    sel = ctx.enter_context(tc.tile_pool(name="sel", bufs=1))
    persist = ctx.enter_context(tc.tile_pool(name="persist", bufs=1))
    hst_pool = ctx.enter_context(tc.tile_pool(name="hst", bufs=3))
    scr_pool = ctx.enter_context(tc.tile_pool(name="scr", bufs=2))
    ps_t = ctx.enter_context(tc.tile_pool(name="ps_t", bufs=2, space="PSUM"))
    ps_acc = ctx.enter_context(tc.tile_pool(name="ps_acc", bufs=2, space="PSUM"))
    ps_bn = ctx.enter_context(tc.tile_pool(name="ps_bn", bufs=2, space="PSUM"))

    # ---- constants ----
    ident = consts.tile([128, 128], f32, tag="ident")
    make_identity(nc, ident[:])
    ones16 = consts.tile([16, 1], f32, tag="ones16")
    nc.gpsimd.memset(ones16[:], 1.0)
    ones_k = consts.tile([K, P], f32, tag="ones_k")
    nc.gpsimd.memset(ones_k[:], 1.0)
    # kidx[p, k] = k + 1 (selection-rank match target)
    kidx_i = consts.tile([128, K], i32, tag="kidx_i")
    nc.gpsimd.iota(kidx_i[:], pattern=[[1, K]], base=1, channel_multiplier=0)
    kidx_f = consts.tile([128, K], f32, tag="kidx_f")
    nc.vector.tensor_copy(out=kidx_f[:], in_=kidx_i[:])
    # pgidx[p, t] = 128*t + p (global patch index per tile column)
    pgidx_i = consts.tile([128, len(PT)], i32, tag="pgidx_i")
    nc.gpsimd.iota(pgidx_i[:], pattern=[[128, len(PT)]], base=0, channel_multiplier=1)
    pgidx_f = consts.tile([128, len(PT)], f32, tag="pgidx_f")
    nc.vector.tensor_copy(out=pgidx_f[:], in_=pgidx_i[:])

    # ---- selection: metric = sum_h attn[h, 0, 1:] ----
    cls = sel.tile([HEADS, P], f32, tag="cls")
    nc.sync.dma_start(out=cls[:], in_=attn[:, 0, 1:])
    met_ps1 = ps_acc.tile([1, 512], f32, tag="acc")
    met_ps2 = ps_acc.tile([1, 64], f32, tag="acc")
    nc.tensor.matmul(met_ps1[:], ones16[:], cls[:, 0:512], start=True, stop=True)
    nc.tensor.matmul(met_ps2[:], ones16[:], cls[:, 512:P], start=True, stop=True)
    metric = sel.tile([1, P], f32, tag="metric")
    nc.vector.tensor_copy(out=metric[0:1, 0:512], in_=met_ps1[:])
    nc.vector.tensor_copy(out=metric[0:1, 512:P], in_=met_ps2[:])

    # ---- region (2x2) argmin -> masked metric ----
    # patch p = (2*ry+dy)*24 + 2*rx+dx  ->  [ry(12), dy(2), rx(12), dx(2)]
    met_r = metric[:].rearrange("p (ry dy rx dx) -> p ry dy rx dx", dy=2, rx=12, dx=2)
    rmin = sel.tile([1, 144], f32, tag="rmin")
    rmin2 = sel.tile([1, 144], f32, tag="rmin2")
    rmin_v = rmin[:].rearrange("p (a b) -> p a b", b=12)
    rmin2_v = rmin2[:].rearrange("p (a b) -> p a b", b=12)
    nc.vector.tensor_tensor(
        out=rmin_v, in0=met_r[:, :, 0, :, 0], in1=met_r[:, :, 0, :, 1], op=Alu.min
    )
    nc.vector.tensor_tensor(
        out=rmin2_v, in0=met_r[:, :, 1, :, 0], in1=met_r[:, :, 1, :, 1], op=Alu.min
    )
    nc.vector.tensor_tensor(out=rmin_v, in0=rmin_v, in1=rmin2_v, op=Alu.min)

    masked = sel.tile([1, P], f32, tag="masked")
    msk_r = masked[:].rearrange("p (ry dy rx dx) -> p ry dy rx dx", dy=2, rx=12, dx=2)
    eq = sel.tile([1, 144], f32, tag="eq")
    eq_v = eq[:].rearrange("p (a b) -> p a b", b=12)
    for dy in range(2):
        for dx in range(2):
            src = met_r[:, :, dy, :, dx]
            nc.vector.tensor_tensor(out=eq_v, in0=src, in1=rmin_v, op=Alu.is_equal)
            # masked = metric + eq * NEG_BIG
            nc.vector.scalar_tensor_tensor(
                out=msk_r[:, :, dy, :, dx],
                in0=eq_v,
                scalar=NEG_BIG,
                in1=src,
                op0=Alu.mult,
                op1=Alu.add,
            )

    # ---- global top-42 threshold (6 rounds of max8 / match_replace) ----
    scratch = sel.tile([1, P], f32, tag="scratch")
    nc.vector.tensor_copy(out=scratch[:], in_=masked[:])
    val48 = sel.tile([1, 48], f32, tag="val48")
    for r in range(6):
        nc.vector.max(out=val48[0:1, 8 * r : 8 * (r + 1)], in_=scratch[:])
        nc.vector.match_replace(
            out=scratch[:],
            in_to_replace=val48[0:1, 8 * r : 8 * (r + 1)],
            in_values=scratch[:],
            imm_value=NEG_BIG,
        )
    # mask of selected patches: masked >= v41 (42nd largest)
    selmask = sel.tile([1, P], f32, tag="selmask")
    nc.vector.tensor_scalar(
        out=selmask[:], in0=masked[:], scalar1=val48[0:1, 41:42], scalar2=None,
        op0=Alu.is_ge,
    )
    # rank of each selected patch: pos = inclusive cumsum(selmask); selected
    # patch with rank r has poskey = r+1, unselected 0.
    pos = sel.tile([1, P], f32, tag="pos")
    nc.vector.tensor_tensor_scan(
        out=pos[:], data0=selmask[:], data1=selmask[:], initial=0.0,
        op0=Alu.add, op1=Alu.bypass,
    )
    poskey = sel.tile([1, P], f32, tag="poskey")
    nc.vector.tensor_tensor(out=poskey[:], in0=pos[:], in1=selmask[:], op=Alu.mult)

    # redistribute poskey to partitions (PE transpose per patch tile), then
    # one-hot O[p, k] = (poskey[p] == k+1)
    o_tiles = []
    for ti, (p0, rows) in enumerate(PT):
        psP = ps_bn.tile([128, 1], f32, tag="ps_bn")
        nc.tensor.transpose(
            psP[0:rows, 0:1], poskey[0:1, p0 : p0 + rows], ident[0:1, 0:1]
        )
        pcol = sel.tile([rows, 1], f32, tag=f"pcol{ti}", name=f"pcol{ti}")
        nc.scalar.copy(out=pcol[:], in_=psP[0:rows, 0:1])
        ot = sel.tile([rows, K], f32, tag=f"oh{ti}", name=f"oh{ti}")
        nc.vector.tensor_scalar(
            out=ot[:], in0=kidx_f[0:rows, :], scalar1=pcol[:], scalar2=None,
            op0=Alu.is_equal,
        )
        o_tiles.append(ot)

    # bench values: bench[k] = sum_p p * O[p, k]  (exact small ints in f32)
    psV = ps_bn.tile([1, K], f32, tag="ps_bn")
    for ti, (p0, rows) in enumerate(PT):
        nc.tensor.matmul(
            psV[:], pgidx_f[0:rows, ti : ti + 1], o_tiles[ti][:],
            start=(ti == 0), stop=(ti == len(PT) - 1),
        )
    bench_i = sel.tile([1, K], i32, tag="bench_i")
    nc.vector.tensor_copy(out=bench_i[:], in_=psV[:])
    nc.sync.dma_start(out=bench_d, in_=bench_i[:])

    # dist_penalty table resident in SBUF (rows gathered via one-hot matmul)
    dist_sb = []
    for ti, (p0, rows) in enumerate(PT):
        dst = persist.tile([rows, P], f32, tag=f"dist{ti}", name=f"dist{ti}")
        nc.sync.dma_start(out=dst[:], in_=dist[p0 : p0 + rows, :])
        dist_sb.append(dst)

    # ---- heavy phase: mean over layers (scale dropped; nrm is scale-inv) ----
    nrm_tiles = []
    agg_tiles = []
    nrmT = [
        persist.tile([128, P], f32, tag=f"nrmT{dt}", name=f"nrmT{dt}")
        for dt in range(ND)
    ]
    for ti, (p0, rows) in enumerate(PT):
        aggt = persist.tile([rows, D], f32, tag=f"agg{ti}")
        nc.sync.dma_start(out=aggt[:], in_=hagg[1 + p0 : 1 + p0 + rows, :])
        agg_tiles.append(aggt)

        # stream 9 layers as two packed halves; one strided reduce each
        LA, LB = 4, 5
        ha = hst_pool.tile([rows, LA * D], f32, tag="hst", name="ha")
        for l in range(LA):
            nc.sync.dma_start(
                out=ha[:, l * D : (l + 1) * D],
                in_=hs[l, 1 + p0 : 1 + p0 + rows, :],
            )
        hb = hst_pool.tile([rows, LB * D], f32, tag="hst", name="hb")
        for l in range(LB):
            nc.sync.dma_start(
                out=hb[:, l * D : (l + 1) * D],
                in_=hs[LA + l, 1 + p0 : 1 + p0 + rows, :],
            )
        pa = scr_pool.tile([rows, D], f32, tag="pa")
        pb = scr_pool.tile([rows, D], f32, tag="pb")
        nc.vector.tensor_reduce(
            out=pa[:], in_=ha[:].rearrange("p (l d) -> p d l", l=LA),
            axis=AX.X, op=Alu.add,
        )
        nc.vector.tensor_reduce(
            out=pb[:], in_=hb[:].rearrange("p (l d) -> p d l", l=LB),
            axis=AX.X, op=Alu.add,
        )
        ssum = scr_pool.tile([rows, D], f32, tag="ssum")
        nc.vector.tensor_add(ssum[:], pa[:], pb[:])

        # l2 normalize rows
        sq = scr_pool.tile([rows, D], f32, tag="sq")
        sumsq = scr_pool.tile([rows, 1], f32, tag="sumsq")
        nc.scalar.activation(out=sq[:], in_=ssum[:], func=Act.Square, accum_out=sumsq[:])
        nrmv = scr_pool.tile([rows, 1], f32, tag="nrmv")
        nc.scalar.activation(out=nrmv[:], in_=sumsq[:], func=Act.Sqrt)
        nc.vector.tensor_scalar_max(nrmv[:], nrmv[:], 1e-12)
        inv = scr_pool.tile([rows, 1], f32, tag="inv")
        nc.vector.reciprocal(out=inv[:], in_=nrmv[:])
        nrmt = persist.tile([rows, D], f32, tag=f"nrm{ti}")
        nc.vector.tensor_scalar(
            out=nrmt[:], in0=ssum[:], scalar1=inv[:], scalar2=None, op0=Alu.mult
        )
        nrm_tiles.append(nrmt)

        # transpose into nrmT d-chunks
        for dt in range(ND):
            psT = ps_t.tile([128, rows], f32, tag="ps_t")
            nc.tensor.transpose(
                psT[:], nrmt[:, 128 * dt : 128 * (dt + 1)], ident[0:rows, 0:rows]
            )
            if dt % 2 == 0:
                nc.vector.tensor_copy(out=nrmT[dt][:, p0 : p0 + rows], in_=psT[:])
            else:
                nc.scalar.copy(out=nrmT[dt][:, p0 : p0 + rows], in_=psT[:])

    # rank of each selected patch: pos = inclusive cumsum(selmask); selected
    # patch with rank r has poskey = r+1, unselected 0.
    pos = sel.tile([1, P], f32, tag="pos")
    nc.vector.tensor_tensor_scan(
        out=pos[:], data0=selmask[:], data1=selmask[:], initial=0.0,
        op0=Alu.add, op1=Alu.bypass,
    )
    poskey = sel.tile([1, P], f32, tag="poskey")
    nc.vector.tensor_tensor(out=poskey[:], in0=pos[:], in1=selmask[:], op=Alu.mult)

    # redistribute poskey to partitions (PE transpose per patch tile), then
    # one-hot O[p, k] = (poskey[p] == k+1)
    o_tiles = []
    for ti, (p0, rows) in enumerate(PT):
        psP = ps_bn.tile([128, 1], f32, tag="ps_bn")
        nc.tensor.transpose(
            psP[0:rows, 0:1], poskey[0:1, p0 : p0 + rows], ident[0:1, 0:1]
        )
        pcol = sel.tile([rows, 1], f32, tag=f"pcol{ti}", name=f"pcol{ti}")
        nc.scalar.copy(out=pcol[:], in_=psP[0:rows, 0:1])
        ot = sel.tile([rows, K], f32, tag=f"oh{ti}", name=f"oh{ti}")
        nc.vector.tensor_scalar(
            out=ot[:], in0=kidx_f[0:rows, :], scalar1=pcol[:], scalar2=None,
            op0=Alu.is_equal,
        )
        o_tiles.append(ot)

    # bench values: bench[k] = sum_p p * O[p, k]  (exact small ints in f32)
    psV = ps_bn.tile([1, K], f32, tag="ps_bn")
    for ti, (p0, rows) in enumerate(PT):
        nc.tensor.matmul(
            psV[:], pgidx_f[0:rows, ti : ti + 1], o_tiles[ti][:],
            start=(ti == 0), stop=(ti == len(PT) - 1),
        )
    bench_i = sel.tile([1, K], i32, tag="bench_i")
    nc.vector.tensor_copy(out=bench_i[:], in_=psV[:])
    nc.sync.dma_start(out=bench_d, in_=bench_i[:])
    # dpen[k, :] = dist_penalty[bench[k], :] via one-hot matmul
    dpen = sel.tile([K, P], f32, tag="dpen")
    dp1 = ps_acc.tile([K, 512], f32, tag="acc")
    dp2 = ps_acc.tile([K, 64], f32, tag="acc")
    for ti, (p0, rows) in enumerate(PT):
        nc.tensor.matmul(
            dp1[:], o_tiles[ti][:], dist_sb[ti][:, 0:512],
            start=(ti == 0), stop=(ti == len(PT) - 1),
        )
        nc.tensor.matmul(
            dp2[:], o_tiles[ti][:], dist_sb[ti][:, 512:P],
            start=(ti == 0), stop=(ti == len(PT) - 1),
        )
    nc.scalar.copy(out=dpen[:, 0:512], in_=dp1[:])
    nc.scalar.copy(out=dpen[:, 512:P], in_=dp2[:])

    # oself[k, p] = O[p, k] (transpose of O)
    oself = sel.tile([K, P], i32, tag="oself")  # int mask for copy_predicated
    for ti, (p0, rows) in enumerate(PT):
        psS = ps_bn.tile([K, 128], f32, tag="ps_bn")
        nc.tensor.transpose(psS[0:K, 0:rows], o_tiles[ti][:], ident[0:rows, 0:rows])
        nc.scalar.copy(out=oself[:, p0 : p0 + rows], in_=psS[0:K, 0:rows])

    # bench_nrm[k, d] = sum_p O[p, k] * nrm[p, d]  (natural layout)
    bnrm = sel.tile([K, D], f32, tag="bnrm")
    bn1 = ps_acc.tile([K, 512], f32, tag="acc")
    bn2 = ps_acc.tile([K, 512], f32, tag="acc")
    for ti, (p0, rows) in enumerate(PT):
        nc.tensor.matmul(
            bn1[:], o_tiles[ti][:], nrm_tiles[ti][:, 0:512],
            start=(ti == 0), stop=(ti == len(PT) - 1),
        )
        nc.tensor.matmul(
            bn2[:], o_tiles[ti][:], nrm_tiles[ti][:, 512:D],
            start=(ti == 0), stop=(ti == len(PT) - 1),
        )
    nc.scalar.copy(out=bnrm[:, 0:512], in_=bn1[:])
    nc.scalar.copy(out=bnrm[:, 512:D], in_=bn2[:])
    # transpose to bench_nrm^T[d-chunk, k]
    bnrmT = []
    for dt in range(ND):
        psBT = ps_bn.tile([128, K], f32, tag="ps_bn")
        nc.tensor.transpose(
            psBT[:], bnrm[:, 128 * dt : 128 * (dt + 1)], ident[0:K, 0:K]
        )
        bt = sel.tile([128, K], f32, tag=f"bnrmT{dt}", name=f"bnrmT{dt}")
        nc.scalar.copy(out=bt[:], in_=psBT[:])
        bnrmT.append(bt)

    # ---- sim = bench_nrm @ nrm^T  [42, 576] ----
    sim1 = ps_acc.tile([K, 512], f32, tag="acc")
    sim2 = ps_acc.tile([K, 64], f32, tag="acc")
    for dt in range(ND):
        nc.tensor.matmul(
            sim1[:], bnrmT[dt][:], nrmT[dt][:, 0:512],
            start=(dt == 0), stop=(dt == ND - 1),
        )
        nc.tensor.matmul(
            sim2[:], bnrmT[dt][:], nrmT[dt][:, 512:P],
            start=(dt == 0), stop=(dt == ND - 1),
        )

    # ---- w = relu(sim) * dpen; normalize; self weight 1.0 ----
    w = sel.tile([K, P], f32, tag="w")
    nc.scalar.activation(out=w[:, 0:512], in_=sim1[:], func=Act.Relu)
    nc.scalar.activation(out=w[:, 512:P], in_=sim2[:], func=Act.Relu)
    nc.vector.tensor_tensor(out=w[:], in0=w[:], in1=dpen[:], op=Alu.mult)
    wsum = sel.tile([K, 1], f32, tag="wsum")
    nc.vector.tensor_reduce(out=wsum[:], in_=w[:], axis=AX.X, op=Alu.add)
    nc.vector.tensor_scalar_add(wsum[:], wsum[:], 1e-8)
    winv = sel.tile([K, 1], f32, tag="winv")
    nc.vector.reciprocal(out=winv[:], in_=wsum[:])
    nc.vector.tensor_scalar(
        out=w[:], in0=w[:], scalar1=winv[:], scalar2=None, op0=Alu.mult
    )
    nc.vector.copy_predicated(out=w[:], mask=oself[:], data=ones_k[:])

    # ---- out = w @ patch_agg  [42, 1024] ----
    wT = []
    for ti, (p0, rows) in enumerate(PT):
        psW = ps_t.tile([128, K], f32, tag="ps_t")
        nc.tensor.transpose(
            psW[0:rows, 0:K], w[:, p0 : p0 + rows], ident[0:K, 0:K]
        )
        wt = sel.tile([rows, K], f32, tag=f"wT{ti}", name=f"wT{ti}")
        nc.scalar.copy(out=wt[:], in_=psW[0:rows, 0:K])
        wT.append(wt)
    o1 = ps_acc.tile([K, 512], f32, tag="acc")
    o2 = ps_acc.tile([K, 512], f32, tag="acc")
    for ti, (p0, rows) in enumerate(PT):
        nc.tensor.matmul(
            o1[:], wT[ti][:], agg_tiles[ti][:, 0:512],
            start=(ti == 0), stop=(ti == len(PT) - 1),
        )
        nc.tensor.matmul(
            o2[:], wT[ti][:], agg_tiles[ti][:, 512:D],
            start=(ti == 0), stop=(ti == len(PT) - 1),
        )
    outsb = sel.tile([K, D], f32, tag="outsb")
    nc.vector.tensor_copy(out=outsb[:, 0:512], in_=o1[:])
    nc.scalar.copy(out=outsb[:, 512:D], in_=o2[:])
    nc.sync.dma_start(out=out_d, in_=outsb[:])


def build():
    from contextlib import ExitStack

    import concourse.bacc as bacc
    from concourse.tile import TileContext

    nc = bacc.Bacc("TRN2")
    with TileContext(nc) as tc:
        with ExitStack() as ctx:
            _emit(nc, tc, ctx)
    nc.compile()
    return nc


_NC_CACHE = {}


def kernel(attn, hidden_agg, stacked_hs):
    import numpy as np

    from concourse.bass_utils import run_bass_kernel_spmd

    if "nc" not in _NC_CACHE:
        _NC_CACHE["nc"] = build()
    nc = _NC_CACHE["nc"]

    dist = _dist_penalty_np()
    in_maps = [
        {
            "attn": np.ascontiguousarray(attn[b]),
            "hidden_agg": np.ascontiguousarray(hidden_agg[b]),
            "stacked_hs": np.ascontiguousarray(stacked_hs[:, b]),
            "dist": dist,
        }
        for b in range(NCORES)
    ]
    res = run_bass_kernel_spmd(nc, in_maps, list(range(NCORES)))
    out = np.stack([res.results[b]["out"] for b in range(NCORES)]).astype(np.float32)
    bench = np.stack(
        [res.results[b]["bench"].reshape(K) for b in range(NCORES)]
    ).astype(np.int32)
    return out, bench


def profile(inputs, tmpdir=None):
    """Run once under NTFF capture; returns HW exec time in ns (or None).

    Leaves the ntff/pftrace artifacts in ``tmpdir`` for trace analysis.
    """
    import glob as _glob
    import os as _os
    import tempfile

    import numpy as np

    from concourse import bass2jax

    try:
        from trn_agent_boot.trn_boot import _ntff_profile_via_ctypes
    except ImportError:
        return None
    hook = _ntff_profile_via_ctypes("/opt/axon/libaxon_pjrt.so")
    if hook is None:
        return None

    if "nc" not in _NC_CACHE:
        _NC_CACHE["nc"] = build()
    nc = _NC_CACHE["nc"]
    dist = _dist_penalty_np()
    in_maps = [
        {
            "attn": np.ascontiguousarray(inputs["attn"][b]),
            "hidden_agg": np.ascontiguousarray(inputs["hidden_agg"][b]),
            "stacked_hs": np.ascontiguousarray(inputs["stacked_hs"][:, b]),
            "dist": dist,
        }
        for b in range(NCORES)
    ]
    tmpdir = tmpdir or tempfile.mkdtemp(prefix="ntffprof_")
    with hook(tmpdir, [0]):
        bass2jax.run_bass_via_pjrt(nc, in_maps, n_cores=NCORES)
    ntffs = _glob.glob(_os.path.join(tmpdir, "*_body*.ntff"))
    print(f"profile dir: {tmpdir} ({len(ntffs)} ntff)")
    if not ntffs:
        return None

    import gauge.profiler
    from concourse._compat import FishPath

    prof = gauge.profiler.Profile(
        profile_path=FishPath(tmpdir),
        kernel_dev_mode=True,
        profile_on_exit=False,
        bass_kernel=nc.m,
        offline_processing=True,
        fname="*_body*",
    )
    try:
        res = prof.to_perfetto(model_index=(0,))
        if res:
            print("trace:", res[0].trace_path)
            return res[0].exec_time_ns
    except Exception as e:
        print(f"to_perfetto failed: {type(e).__name__}: {e}")
    return None


# revision 12
# speedup vs baseline: 1.0548x; 1.0548x over previous
"""Trainium2 Bass kernel for nn_CLIPVisionTower_Nuwa_abli (topk_masking).

Per-image pipeline (pure batch data-parallel, 1 image per NeuronCore):
  metric  = sum over heads of attn[:, 0, 1:]                  [576]
  mask out each 2x2-region argmin, take global top-42 of the
  remainder, bench = those patch indices ascending             [42]
  nrm     = l2-normalized mean over 9 layers of hs[:, 1:, :]   [576,1024]
  sim     = nrm[bench] @ nrm.T                                 [42,576]
  w       = relu(sim) * dist_penalty[bench]; row-normalize; self weight 1
  out     = w @ hidden_agg[1:, :]                              [42,1024]
"""

import math

import numpy as np

B = 8
HGRID = 24
P = 576  # patches
K = 42  # bench tokens
HEADS = 16
D = 1024
L = 9  # layers
NCORES = 8
PT = [(0, 128), (128, 128), (256, 128), (384, 128), (512, 64)]  # patch tiles
ND = D // 128  # 8 d-chunks of 128
NEG_BIG = -1.0e30
KEY_BIG = 1.0e9


def _dist_penalty_np() -> np.ndarray:
    ys, xs = np.meshgrid(
        np.arange(HGRID, dtype=np.float32),
        np.arange(HGRID, dtype=np.float32),
        indexing="ij",
    )
    coords = np.stack([ys, xs], axis=-1).reshape(-1, 2)
    diff = coords[:, None, :] - coords[None, :, :]
    dist = np.sqrt((diff * diff).sum(axis=-1), dtype=np.float32)
    thresh = np.float32(math.sqrt(280.0))
    return (np.float32(1.0) - np.minimum(dist / thresh, np.float32(1.0))).astype(
        np.float32
    )


def _emit(nc, tc, ctx):
    import concourse.mybir as mybir
    from concourse.masks import make_identity

    f32 = mybir.dt.float32
    i32 = mybir.dt.int32
    Alu = mybir.AluOpType
    Act = mybir.ActivationFunctionType
    AX = mybir.AxisListType
    NT = len(PT)

    attn = nc.dram_tensor("attn", [HEADS, P + 1, P + 1], f32, kind="ExternalInput").ap()
    hagg = nc.dram_tensor("hidden_agg", [P + 1, D], f32, kind="ExternalInput").ap()
    hs = nc.dram_tensor("stacked_hs", [L, P + 1, D], f32, kind="ExternalInput").ap()
    dist = nc.dram_tensor("dist", [P, P], f32, kind="ExternalInput").ap()
    out_d = nc.dram_tensor("out", [K, D], f32, kind="ExternalOutput").ap()
    bench_d = nc.dram_tensor("bench", [1, K], i32, kind="ExternalOutput").ap()

    consts = ctx.enter_context(tc.tile_pool(name="consts", bufs=1))
    sel = ctx.enter_context(tc.tile_pool(name="sel", bufs=1))
    persist = ctx.enter_context(tc.tile_pool(name="persist", bufs=1))
    hst_pool = ctx.enter_context(tc.tile_pool(name="hst", bufs=2))
    scr_pool = ctx.enter_context(tc.tile_pool(name="scr", bufs=2))
    ps_t = ctx.enter_context(tc.tile_pool(name="ps_t", bufs=2, space="PSUM"))
    ps_acc = ctx.enter_context(tc.tile_pool(name="ps_acc", bufs=2, space="PSUM"))
    ps_bn = ctx.enter_context(tc.tile_pool(name="ps_bn", bufs=2, space="PSUM"))

    # ---- constants ----
    ident = consts.tile([128, 128], f32, tag="ident")
    make_identity(nc, ident[:])
    ones16 = consts.tile([16, 1], f32, tag="ones16")
    nc.gpsimd.memset(ones16[:], 1.0)
    ones_r = consts.tile([1, 128], f32, tag="ones_r")
    nc.gpsimd.memset(ones_r[:], 1.0)
    ones_c = consts.tile([128, 1], f32, tag="ones_c")
    nc.gpsimd.memset(ones_c[:], 1.0)
    ones_sm = consts.tile([128, K], f32, tag="ones_sm")
    nc.gpsimd.memset(ones_sm[:], 1.0)
    # kidx[p, k] = k + 1 (selection-rank match target)
    kidx_i = consts.tile([128, K], i32, tag="kidx_i")
    nc.gpsimd.iota(kidx_i[:], pattern=[[1, K]], base=1, channel_multiplier=0)
    kidx_f = consts.tile([128, K], f32, tag="kidx_f")
    nc.vector.tensor_copy(out=kidx_f[:], in_=kidx_i[:])
    # pgidx[p, t] = 128*t + p (global patch index per tile column)
    pgidx_i = consts.tile([128, NT], i32, tag="pgidx_i")
    nc.gpsimd.iota(pgidx_i[:], pattern=[[128, NT]], base=0, channel_multiplier=1)
    pgidx_f = consts.tile([128, NT], f32, tag="pgidx_f")
    nc.vector.tensor_copy(out=pgidx_f[:], in_=pgidx_i[:])

    # ---- selection: metric = sum_h attn[h, 0, 1:] ----
    cls = sel.tile([HEADS, P], f32, tag="cls")
    nc.sync.dma_start(out=cls[:], in_=attn[:, 0, 1:])
    met_ps1 = ps_acc.tile([1, 512], f32, tag="acc")
    met_ps2 = ps_acc.tile([1, 64], f32, tag="acc")
    nc.tensor.matmul(met_ps1[:], ones16[:], cls[:, 0:512], start=True, stop=True)
    nc.tensor.matmul(met_ps2[:], ones16[:], cls[:, 512:P], start=True, stop=True)
    metric = sel.tile([1, P], f32, tag="metric")
    nc.vector.tensor_copy(out=metric[0:1, 0:512], in_=met_ps1[:])
    nc.vector.tensor_copy(out=metric[0:1, 512:P], in_=met_ps2[:])

    # ---- region (2x2) argmin -> masked metric ----
    # patch p = (2*ry+dy)*24 + 2*rx+dx  ->  [ry(12), dy(2), rx(12), dx(2)]
    met_r = metric[:].rearrange("p (ry dy rx dx) -> p ry dy rx dx", dy=2, rx=12, dx=2)
    rmin = sel.tile([1, 144], f32, tag="rmin")
    rmin2 = sel.tile([1, 144], f32, tag="rmin2")
    rmin_v = rmin[:].rearrange("p (a b) -> p a b", b=12)
    rmin2_v = rmin2[:].rearrange("p (a b) -> p a b", b=12)
    nc.vector.tensor_tensor(
        out=rmin_v, in0=met_r[:, :, 0, :, 0], in1=met_r[:, :, 0, :, 1], op=Alu.min
    )
    nc.vector.tensor_tensor(
        out=rmin2_v, in0=met_r[:, :, 1, :, 0], in1=met_r[:, :, 1, :, 1], op=Alu.min
    )
    nc.vector.tensor_tensor(out=rmin_v, in0=rmin_v, in1=rmin2_v, op=Alu.min)

    masked = sel.tile([1, P], f32, tag="masked")
    msk_r = masked[:].rearrange("p (ry dy rx dx) -> p ry dy rx dx", dy=2, rx=12, dx=2)
    eq = sel.tile([1, 144], f32, tag="eq")
    eq_v = eq[:].rearrange("p (a b) -> p a b", b=12)
    for dy in range(2):
        for dx in range(2):
            src = met_r[:, :, dy, :, dx]
            nc.vector.tensor_tensor(out=eq_v, in0=src, in1=rmin_v, op=Alu.is_equal)
            # masked = metric + eq * NEG_BIG
            nc.vector.scalar_tensor_tensor(
                out=msk_r[:, :, dy, :, dx],
                in0=eq_v,
                scalar=NEG_BIG,
                in1=src,
                op0=Alu.mult,
                op1=Alu.add,
            )

    # ---- global top-42 threshold (6 rounds of max8 / match_replace) ----
    scratch = sel.tile([1, P], f32, tag="scratch")
    nc.vector.tensor_copy(out=scratch[:], in_=masked[:])
    val48 = sel.tile([1, 48], f32, tag="val48")
    for r in range(6):
        nc.vector.max(out=val48[0:1, 8 * r : 8 * (r + 1)], in_=scratch[:])
        nc.vector.match_replace(
            out=scratch[:],
            in_to_replace=val48[0:1, 8 * r : 8 * (r + 1)],
            in_values=scratch[:],
            imm_value=NEG_BIG,
        )
    # mask of selected patches: masked >= v41 (42nd largest)
    selmask = sel.tile([1, P], f32, tag="selmask")
    nc.vector.tensor_scalar(
        out=selmask[:], in0=masked[:], scalar1=val48[0:1, 41:42], scalar2=None,
        op0=Alu.is_ge,
    )
    # rank of each selected patch: pos = inclusive cumsum(selmask); selected
    # patch with rank r has poskey = r+1, unselected 0.
    pos = sel.tile([1, P], f32, tag="pos")
    nc.vector.tensor_tensor_scan(
        out=pos[:], data0=selmask[:], data1=selmask[:], initial=0.0,
        op0=Alu.add, op1=Alu.bypass,
    )
    poskey = sel.tile([1, P], f32, tag="poskey")
    nc.vector.tensor_tensor(out=poskey[:], in0=pos[:], in1=selmask[:], op=Alu.mult)

    # redistribute poskey to partitions (PE transpose per patch tile), then
    # one-hot O[p, k] = (poskey[p] == k+1); int copy doubles as self-mask
    o_tiles = []
    oi_tiles = []
    for ti, (p0, rows) in enumerate(PT):
        psP = ps_bn.tile([128, 1], f32, tag="ps_bn")
        nc.tensor.transpose(
            psP[0:rows, 0:1], poskey[0:1, p0 : p0 + rows], ident[0:1, 0:1]
        )
        pcol = sel.tile([rows, 1], f32, tag=f"pcol{ti}", name=f"pcol{ti}")
        nc.scalar.copy(out=pcol[:], in_=psP[0:rows, 0:1])
        ot = sel.tile([rows, K], f32, tag=f"oh{ti}", name=f"oh{ti}")
        nc.vector.tensor_scalar(
            out=ot[:], in0=kidx_f[0:rows, :], scalar1=pcol[:], scalar2=None,
            op0=Alu.is_equal,
        )
        o_tiles.append(ot)
        oi = sel.tile([rows, K], i32, tag=f"ohi{ti}", name=f"ohi{ti}")
        nc.vector.tensor_copy(out=oi[:], in_=ot[:])
        oi_tiles.append(oi)

    # bench values: bench[k] = sum_p p * O[p, k]  (exact small ints in f32)
    psV = ps_bn.tile([1, K], f32, tag="ps_bn")
    for ti, (p0, rows) in enumerate(PT):
        nc.tensor.matmul(
            psV[:], pgidx_f[0:rows, ti : ti + 1], o_tiles[ti][:],
            start=(ti == 0), stop=(ti == NT - 1),
        )
    bench_i = sel.tile([1, K], i32, tag="bench_i")
    nc.vector.tensor_copy(out=bench_i[:], in_=psV[:])
    nc.sync.dma_start(out=bench_d, in_=bench_i[:])

    # dist_penalty table resident in SBUF; dpenT[p, k] = dist[p, bench[k]]
    # (dist is symmetric) via dpenT = dist @ O
    dist_sb = []
    for ti, (p0, rows) in enumerate(PT):
        dst = persist.tile([rows, P], f32, tag=f"dist{ti}", name=f"dist{ti}")
        nc.sync.dma_start(out=dst[:], in_=dist[p0 : p0 + rows, :])
        dist_sb.append(dst)
    dpT = []
    for tp, (pp0, prows) in enumerate(PT):
        psD = ps_bn.tile([128, K], f32, tag="ps_bn")
        for tq, (q0, qrows) in enumerate(PT):
            nc.tensor.matmul(
                psD[0:prows, :],
                dist_sb[tq][:, pp0 : pp0 + prows],
                o_tiles[tq][:],
                start=(tq == 0),
                stop=(tq == NT - 1),
            )
        dpt = sel.tile([prows, K], f32, tag=f"dpT{tp}", name=f"dpT{tp}")
        nc.scalar.copy(out=dpt[:], in_=psD[0:prows, :])
        dpT.append(dpt)

    # ---- heavy phase: layer sum (scale dropped; nrm is scale-invariant) ----
    nrm_tiles = []
    agg_tiles = []
    nrmT = [
        persist.tile([128, P], f32, tag=f"nrmT{dt}", name=f"nrmT{dt}")
        for dt in range(ND)
    ]
    LA, LB = 4, 5
    for ti, (p0, rows) in enumerate(PT):
        aggt = persist.tile([rows, D], f32, tag=f"agg{ti}", name=f"agg{ti}")
        nc.sync.dma_start(out=aggt[:], in_=hagg[1 + p0 : 1 + p0 + rows, :])
        agg_tiles.append(aggt)

        # stream 9 layers as two packed halves; contiguous binary-tree adds
        # split between DVE and GpSimd
        ha = hst_pool.tile([rows, LA * D], f32, tag="ha", name="ha")
        for l in range(LA):
            nc.sync.dma_start(
                out=ha[:, l * D : (l + 1) * D],
                in_=hs[l, 1 + p0 : 1 + p0 + rows, :],
            )
        hb = hst_pool.tile([rows, LB * D], f32, tag="hb", name="hb")
        for l in range(LB):
            nc.sync.dma_start(
                out=hb[:, l * D : (l + 1) * D],
                in_=hs[LA + l, 1 + p0 : 1 + p0 + rows, :],
            )
        nc.vector.tensor_add(ha[:, 0 : 2 * D], ha[:, 0 : 2 * D], ha[:, 2 * D : 4 * D])
        nc.gpsimd.tensor_add(hb[:, 0 : 2 * D], hb[:, 0 : 2 * D], hb[:, 2 * D : 4 * D])
        nc.vector.tensor_add(ha[:, 0:D], ha[:, 0:D], ha[:, D : 2 * D])
        nc.gpsimd.tensor_add(hb[:, 0:D], hb[:, 0:D], hb[:, D : 2 * D])
        nc.vector.tensor_add(ha[:, 0:D], ha[:, 0:D], hb[:, 0:D])
        ssum = scr_pool.tile([rows, D], f32, tag="ssum")
        nc.vector.tensor_add(ssum[:], ha[:, 0:D], hb[:, 4 * D : 5 * D])

        # l2 normalize rows
        sq = scr_pool.tile([rows, D], f32, tag="sq", bufs=1)
        sumsq = scr_pool.tile([rows, 1], f32, tag="sumsq")
        nc.scalar.activation(out=sq[:], in_=ssum[:], func=Act.Square, accum_out=sumsq[:])
        nrmv = scr_pool.tile([rows, 1], f32, tag="nrmv")
        nc.scalar.activation(out=nrmv[:], in_=sumsq[:], func=Act.Sqrt)
        nc.vector.tensor_scalar_max(nrmv[:], nrmv[:], 1e-12)
        inv = scr_pool.tile([rows, 1], f32, tag="inv")
        nc.vector.reciprocal(out=inv[:], in_=nrmv[:])
        nrmt = persist.tile([rows, D], f32, tag=f"nrm{ti}", name=f"nrm{ti}")
        nc.vector.tensor_scalar(
            out=nrmt[:], in0=ssum[:], scalar1=inv[:], scalar2=None, op0=Alu.mult
        )
        nrm_tiles.append(nrmt)

        # transpose into nrmT d-chunks
        for dt in range(ND):
            psT = ps_t.tile([128, rows], f32, tag="ps_t")
            nc.tensor.transpose(
                psT[:], nrmt[:, 128 * dt : 128 * (dt + 1)], ident[0:rows, 0:rows]
            )
            if dt % 2 == 0:
                nc.vector.tensor_copy(out=nrmT[dt][:, p0 : p0 + rows], in_=psT[:])
            else:
                nc.scalar.copy(out=nrmT[dt][:, p0 : p0 + rows], in_=psT[:])

    # ---- bench_nrm^T[d, k] = sum_p nrm[p, d] * O[p, k] ----
    bnrmT = []
    for dt in range(ND):
        psB = ps_bn.tile([128, K], f32, tag="ps_bn")
        for ti, (p0, rows) in enumerate(PT):
            nc.tensor.matmul(
                psB[:],
                nrm_tiles[ti][:, 128 * dt : 128 * (dt + 1)],
                o_tiles[ti][:],
                start=(ti == 0),
                stop=(ti == NT - 1),
            )
        bt = sel.tile([128, K], f32, tag=f"bnrmT{dt}", name=f"bnrmT{dt}")
        nc.scalar.copy(out=bt[:], in_=psB[:])
        bnrmT.append(bt)

    # ---- simT[p, k] = sum_d nrm[p, d] bnrm[k, d]; w kept transposed ----
    wT = []
    for ti, (p0, rows) in enumerate(PT):
        psS = ps_bn.tile([128, K], f32, tag="ps_bn")
        for dt in range(ND):
            nc.tensor.matmul(
                psS[0:rows, :],
                nrmT[dt][:, p0 : p0 + rows],
                bnrmT[dt][:],
                start=(dt == 0),
                stop=(dt == ND - 1),
            )
        wt = sel.tile([rows, K], f32, tag=f"wT{ti}", name=f"wT{ti}")
        nc.scalar.activation(out=wt[:], in_=psS[0:rows, :], func=Act.Relu)
        nc.vector.tensor_tensor(out=wt[:], in0=wt[:], in1=dpT[ti][:], op=Alu.mult)
        wT.append(wt)

    # row sums over p (partitions) via ones matmul -> [1, 42]
    psW = ps_bn.tile([1, K], f32, tag="ps_bn")
    for ti, (p0, rows) in enumerate(PT):
        nc.tensor.matmul(
            psW[:], ones_c[0:rows, :], wT[ti][:],
            start=(ti == 0), stop=(ti == NT - 1),
        )
    wsum = sel.tile([1, K], f32, tag="wsum")
    nc.vector.tensor_copy(out=wsum[:], in_=psW[:])
    nc.vector.tensor_scalar_add(wsum[:], wsum[:], 1e-8)
    winv = sel.tile([1, K], f32, tag="winv")
    nc.vector.reciprocal(out=winv[:], in_=wsum[:])
    psWB = ps_bn.tile([128, K], f32, tag="ps_bn")
    nc.tensor.matmul(psWB[:], ones_r[:], winv[:], start=True, stop=True)
    winvb = sel.tile([128, K], f32, tag="winvb")
    nc.scalar.copy(out=winvb[:], in_=psWB[:])
    for ti, (p0, rows) in enumerate(PT):
        nc.vector.tensor_tensor(
            out=wT[ti][:], in0=wT[ti][:], in1=winvb[0:rows, :], op=Alu.mult
        )
        nc.vector.copy_predicated(
            out=wT[ti][:], mask=oi_tiles[ti][:], data=ones_sm[0:rows, :]
        )

    # ---- out = w @ patch_agg  [42, 1024] ----
    o1 = ps_acc.tile([K, 512], f32, tag="acc")
    o2 = ps_acc.tile([K, 512], f32, tag="acc")
    for ti, (p0, rows) in enumerate(PT):
        nc.tensor.matmul(
            o1[:], wT[ti][:], agg_tiles[ti][:, 0:512],
            start=(ti == 0), stop=(ti == NT - 1),
        )
        nc.tensor.matmul(
            o2[:], wT[ti][:], agg_tiles[ti][:, 512:D],
            start=(ti == 0), stop=(ti == NT - 1),
        )
    outsb = sel.tile([K, D], f32, tag="outsb")
    nc.vector.tensor_copy(out=outsb[:, 0:512], in_=o1[:])
    nc.scalar.copy(out=outsb[:, 512:D], in_=o2[:])
    nc.sync.dma_start(out=out_d, in_=outsb[:])


def build():
    from contextlib import ExitStack

    import concourse.bacc as bacc
    from concourse.tile import TileContext

    nc = bacc.Bacc("TRN2")
    with TileContext(nc) as tc:
        with ExitStack() as ctx:
            _emit(nc, tc, ctx)
    nc.compile()
    return nc


_NC_CACHE = {}


def kernel(attn, hidden_agg, stacked_hs):
    import numpy as np

    from concourse.bass_utils import run_bass_kernel_spmd

    if "nc" not in _NC_CACHE:
        _NC_CACHE["nc"] = build()
    nc = _NC_CACHE["nc"]

    dist = _dist_penalty_np()
    in_maps = [
        {
            "attn": np.ascontiguousarray(attn[b]),
            "hidden_agg": np.ascontiguousarray(hidden_agg[b]),
            "stacked_hs": np.ascontiguousarray(stacked_hs[:, b]),
            "dist": dist,
        }
        for b in range(NCORES)
    ]
    res = run_bass_kernel_spmd(nc, in_maps, list(range(NCORES)))
    out = np.stack([res.results[b]["out"] for b in range(NCORES)]).astype(np.float32)
    bench = np.stack(
        [res.results[b]["bench"].reshape(K) for b in range(NCORES)]
    ).astype(np.int32)
    return out, bench


def profile(inputs, tmpdir=None):
    """Run once under NTFF capture; returns HW exec time in ns (or None).

    Leaves the ntff/pftrace artifacts in ``tmpdir`` for trace analysis.
    """
    import glob as _glob
    import os as _os
    import tempfile

    import numpy as np

    from concourse import bass2jax

    try:
        from trn_agent_boot.trn_boot import _ntff_profile_via_ctypes
    except ImportError:
        return None
    hook = _ntff_profile_via_ctypes("/opt/axon/libaxon_pjrt.so")
    if hook is None:
        return None

    if "nc" not in _NC_CACHE:
        _NC_CACHE["nc"] = build()
    nc = _NC_CACHE["nc"]
    dist = _dist_penalty_np()
    in_maps = [
        {
            "attn": np.ascontiguousarray(inputs["attn"][b]),
            "hidden_agg": np.ascontiguousarray(inputs["hidden_agg"][b]),
            "stacked_hs": np.ascontiguousarray(inputs["stacked_hs"][:, b]),
            "dist": dist,
        }
        for b in range(NCORES)
    ]
    tmpdir = tmpdir or tempfile.mkdtemp(prefix="ntffprof_")
    with hook(tmpdir, [0]):
        bass2jax.run_bass_via_pjrt(nc, in_maps, n_cores=NCORES)
    ntffs = _glob.glob(_os.path.join(tmpdir, "*_body*.ntff"))
    print(f"profile dir: {tmpdir} ({len(ntffs)} ntff)")
    if not ntffs:
        return None

    import gauge.profiler
    from concourse._compat import FishPath

    prof = gauge.profiler.Profile(
        profile_path=FishPath(tmpdir),
        kernel_dev_mode=True,
        profile_on_exit=False,
        bass_kernel=nc.m,
        offline_processing=True,
        fname="*_body*",
    )
    try:
        res = prof.to_perfetto(model_index=(0,))
        if res:
            print("trace:", res[0].trace_path)
            return res[0].exec_time_ns
    except Exception as e:
        print(f"to_perfetto failed: {type(e).__name__}: {e}")
    return None


# revision 13
# speedup vs baseline: 1.2229x; 1.1595x over previous
"""Trainium2 Bass kernel for nn_CLIPVisionTower_Nuwa_abli (topk_masking).

Per-image pipeline (pure batch data-parallel, 1 image per NeuronCore):
  metric  = sum over heads of attn[:, 0, 1:]                  [576]
  mask out each 2x2-region argmin, take global top-42 of the
  remainder, bench = those patch indices ascending             [42]
  nrm     = l2-normalized mean over 9 layers of hs[:, 1:, :]   [576,1024]
  sim     = nrm[bench] @ nrm.T                                 [42,576]
  w       = relu(sim) * dist_penalty[bench]; row-normalize; self weight 1
  out     = w @ hidden_agg[1:, :]                              [42,1024]
"""

import math

import numpy as np

B = 8
HGRID = 24
P = 576  # patches
K = 42  # bench tokens
HEADS = 16
D = 1024
L = 9  # layers
NCORES = 8
PT = [(0, 128), (128, 128), (256, 128), (384, 128), (512, 64)]  # patch tiles
ND = D // 128  # 8 d-chunks of 128
NEG_BIG = -1.0e30
KEY_BIG = 1.0e9


def _dist_penalty_np() -> np.ndarray:
    ys, xs = np.meshgrid(
        np.arange(HGRID, dtype=np.float32),
        np.arange(HGRID, dtype=np.float32),
        indexing="ij",
    )
    coords = np.stack([ys, xs], axis=-1).reshape(-1, 2)
    diff = coords[:, None, :] - coords[None, :, :]
    dist = np.sqrt((diff * diff).sum(axis=-1), dtype=np.float32)
    thresh = np.float32(math.sqrt(280.0))
    return (np.float32(1.0) - np.minimum(dist / thresh, np.float32(1.0))).astype(
        np.float32
    )


def _emit(nc, tc, ctx):
    import concourse.bass as bass
    import concourse.mybir as mybir
    from concourse.masks import make_identity

    f32 = mybir.dt.float32
    i32 = mybir.dt.int32
    Alu = mybir.AluOpType
    Act = mybir.ActivationFunctionType
    AX = mybir.AxisListType
    NT = len(PT)

    attn = nc.dram_tensor("attn", [HEADS, P + 1, P + 1], f32, kind="ExternalInput").ap()
    hagg = nc.dram_tensor("hidden_agg", [P + 1, D], f32, kind="ExternalInput").ap()
    hs = nc.dram_tensor("stacked_hs", [L, P + 1, D], f32, kind="ExternalInput").ap()
    dist = nc.dram_tensor("dist", [P, P], f32, kind="ExternalInput").ap()
    out_d = nc.dram_tensor("out", [K, D], f32, kind="ExternalOutput").ap()
    bench_d = nc.dram_tensor("bench", [1, K], i32, kind="ExternalOutput").ap()

    consts = ctx.enter_context(tc.tile_pool(name="consts", bufs=1))
    sel = ctx.enter_context(tc.tile_pool(name="sel", bufs=1))
    persist = ctx.enter_context(tc.tile_pool(name="persist", bufs=1))
    hst_pool = ctx.enter_context(tc.tile_pool(name="hst", bufs=2))
    scr_pool = ctx.enter_context(tc.tile_pool(name="scr", bufs=2))
    ps_t = ctx.enter_context(tc.tile_pool(name="ps_t", bufs=2, space="PSUM"))
    ps_acc = ctx.enter_context(tc.tile_pool(name="ps_acc", bufs=2, space="PSUM"))
    ps_bn = ctx.enter_context(tc.tile_pool(name="ps_bn", bufs=2, space="PSUM"))

    # ---- constants ----
    ident = consts.tile([128, 128], f32, tag="ident")
    make_identity(nc, ident[:])
    ones16 = consts.tile([16, 1], f32, tag="ones16")
    nc.gpsimd.memset(ones16[:], 1.0)
    ones_c = consts.tile([128, 1], f32, tag="ones_c")
    nc.gpsimd.memset(ones_c[:], 1.0)
    # kidx[p, k] = k + 1 (selection-rank match target)
    kidx_i = consts.tile([128, K], i32, tag="kidx_i")
    nc.gpsimd.iota(kidx_i[:], pattern=[[1, K]], base=1, channel_multiplier=0)
    kidx_f = consts.tile([128, K], f32, tag="kidx_f")
    nc.vector.tensor_copy(out=kidx_f[:], in_=kidx_i[:])
    # pgidx[p, t] = 128*t + p (global patch index per tile column)
    pgidx_i = consts.tile([128, NT], i32, tag="pgidx_i")
    nc.gpsimd.iota(pgidx_i[:], pattern=[[128, NT]], base=0, channel_multiplier=1)
    pgidx_f = consts.tile([128, NT], f32, tag="pgidx_f")
    nc.vector.tensor_copy(out=pgidx_f[:], in_=pgidx_i[:])

    # ---- selection: metric = sum_h attn[h, 0, 1:] ----
    cls = sel.tile([HEADS, P], f32, tag="cls")
    nc.sync.dma_start(out=cls[:], in_=attn[:, 0, 1:])
    met_ps1 = ps_acc.tile([1, 512], f32, tag="acc")
    met_ps2 = ps_acc.tile([1, 64], f32, tag="acc")
    nc.tensor.matmul(met_ps1[:], ones16[:], cls[:, 0:512], start=True, stop=True)
    nc.tensor.matmul(met_ps2[:], ones16[:], cls[:, 512:P], start=True, stop=True)
    metric = sel.tile([1, P], f32, tag="metric")
    nc.vector.tensor_copy(out=metric[0:1, 0:512], in_=met_ps1[:])
    nc.vector.tensor_copy(out=metric[0:1, 512:P], in_=met_ps2[:])

    # ---- region (2x2) argmin -> masked metric ----
    # patch p = (2*ry+dy)*24 + 2*rx+dx  ->  [ry(12), dy(2), rx(12), dx(2)]
    met_r = metric[:].rearrange("p (ry dy rx dx) -> p ry dy rx dx", dy=2, rx=12, dx=2)
    rmin = sel.tile([1, 144], f32, tag="rmin")
    rmin2 = sel.tile([1, 144], f32, tag="rmin2")
    rmin_v = rmin[:].rearrange("p (a b) -> p a b", b=12)
    rmin2_v = rmin2[:].rearrange("p (a b) -> p a b", b=12)
    nc.vector.tensor_tensor(
        out=rmin_v, in0=met_r[:, :, 0, :, 0], in1=met_r[:, :, 0, :, 1], op=Alu.min
    )
    nc.vector.tensor_tensor(
        out=rmin2_v, in0=met_r[:, :, 1, :, 0], in1=met_r[:, :, 1, :, 1], op=Alu.min
    )
    nc.vector.tensor_tensor(out=rmin_v, in0=rmin_v, in1=rmin2_v, op=Alu.min)

    masked = sel.tile([1, P], f32, tag="masked")
    msk_r = masked[:].rearrange("p (ry dy rx dx) -> p ry dy rx dx", dy=2, rx=12, dx=2)
    eq = sel.tile([1, 144], f32, tag="eq")
    eq_v = eq[:].rearrange("p (a b) -> p a b", b=12)
    for dy in range(2):
        for dx in range(2):
            src = met_r[:, :, dy, :, dx]
            nc.vector.tensor_tensor(out=eq_v, in0=src, in1=rmin_v, op=Alu.is_equal)
            # masked = metric + eq * NEG_BIG
            nc.vector.scalar_tensor_tensor(
                out=msk_r[:, :, dy, :, dx],
                in0=eq_v,
                scalar=NEG_BIG,
                in1=src,
                op0=Alu.mult,
                op1=Alu.add,
            )

    # ---- global top-42 threshold (6 rounds of max8 / match_replace) ----
    scratch = sel.tile([1, P], f32, tag="scratch")
    nc.vector.tensor_copy(out=scratch[:], in_=masked[:])
    val48 = sel.tile([1, 48], f32, tag="val48")
    for r in range(6):
        nc.vector.max(out=val48[0:1, 8 * r : 8 * (r + 1)], in_=scratch[:])
        nc.vector.match_replace(
            out=scratch[:],
            in_to_replace=val48[0:1, 8 * r : 8 * (r + 1)],
            in_values=scratch[:],
            imm_value=NEG_BIG,
        )
    # mask of selected patches: masked >= v41 (42nd largest)
    selmask = sel.tile([1, P], f32, tag="selmask")
    nc.vector.tensor_scalar(
        out=selmask[:], in0=masked[:], scalar1=val48[0:1, 41:42], scalar2=None,
        op0=Alu.is_ge,
    )
    # rank of each selected patch: pos = inclusive cumsum(selmask); selected
    # patch with rank r has poskey = r+1, unselected 0.
    pos = sel.tile([1, P], f32, tag="pos")
    nc.vector.tensor_tensor_scan(
        out=pos[:], data0=selmask[:], data1=selmask[:], initial=0.0,
        op0=Alu.add, op1=Alu.bypass,
    )
    poskey = sel.tile([1, P], f32, tag="poskey")
    nc.vector.tensor_tensor(out=poskey[:], in0=pos[:], in1=selmask[:], op=Alu.mult)

    # redistribute poskey to partitions (PE transpose per patch tile), then
    # one-hot O[p, k] = (poskey[p] == k+1)
    o_tiles = []
    for ti, (p0, rows) in enumerate(PT):
        psP = ps_bn.tile([128, 1], f32, tag="ps_bn")
        nc.tensor.transpose(
            psP[0:rows, 0:1], poskey[0:1, p0 : p0 + rows], ident[0:1, 0:1]
        )
        pcol = sel.tile([rows, 1], f32, tag=f"pcol{ti}", name=f"pcol{ti}")
        nc.scalar.copy(out=pcol[:], in_=psP[0:rows, 0:1])
        ot = sel.tile([rows, K], f32, tag=f"oh{ti}", name=f"oh{ti}")
        nc.vector.tensor_scalar(
            out=ot[:], in0=kidx_f[0:rows, :], scalar1=pcol[:], scalar2=None,
            op0=Alu.is_equal,
        )
        o_tiles.append(ot)

    # bench values: bench[k] = sum_p p * O[p, k]  (exact small ints in f32)
    psV = ps_bn.tile([1, K], f32, tag="ps_bn")
    for ti, (p0, rows) in enumerate(PT):
        nc.tensor.matmul(
            psV[:], pgidx_f[0:rows, ti : ti + 1], o_tiles[ti][:],
            start=(ti == 0), stop=(ti == NT - 1),
        )
    bench_i = sel.tile([1, K], i32, tag="bench_i")
    nc.vector.tensor_copy(out=bench_i[:], in_=psV[:])
    nc.sync.dma_start(out=bench_d, in_=bench_i[:])

    # bench as an int32 column (one index per partition) for indirect gathers
    benchfr = sel.tile([1, K], f32, tag="benchfr")
    nc.scalar.copy(out=benchfr[:], in_=psV[:])
    psC = ps_bn.tile([128, 1], f32, tag="ps_bn")
    nc.tensor.transpose(psC[0:K, 0:1], benchfr[:], ident[0:1, 0:1])
    benchcol = sel.tile([K, 1], i32, tag="benchcol")
    nc.vector.tensor_copy(out=benchcol[:], in_=psC[0:K, 0:1])

    # dpen[k, :] = dist_penalty[bench[k], :] via indirect row gather
    dpen = sel.tile([K, P], f32, tag="dpen")
    nc.gpsimd.indirect_dma_start(
        out=dpen[:],
        out_offset=None,
        in_=dist,
        in_offset=bass.IndirectOffsetOnAxis(ap=benchcol[:, 0:1], axis=0),
    )

    # oself[k, p] = O[p, k] (transpose of O; int mask for copy_predicated)
    oself = sel.tile([K, P], i32, tag="oself")
    for ti, (p0, rows) in enumerate(PT):
        psS = ps_t.tile([K, 128], f32, tag="ps_t")
        nc.tensor.transpose(psS[0:K, 0:rows], o_tiles[ti][:], ident[0:rows, 0:rows])
        nc.vector.tensor_copy(out=oself[:, p0 : p0 + rows], in_=psS[0:K, 0:rows])

    # ---- heavy phase: layer sum (scale dropped; nrm is scale-invariant) ----
    nrm_tiles = []
    nrmT = [
        persist.tile([128, P], f32, tag=f"nrmT{dt}", name=f"nrmT{dt}")
        for dt in range(ND)
    ]
    bnrmT = [
        sel.tile([128, K], f32, tag=f"bnrmT{dt}", name=f"bnrmT{dt}")
        for dt in range(ND)
    ]
    LA, LB = 4, 5
    for ti, (p0, rows) in enumerate(PT):
        # stream 9 layers as two packed halves; contiguous binary-tree adds
        # split between DVE and GpSimd, folding into dead tile regions
        ha = hst_pool.tile([rows, LA * D], f32, tag="ha", name="ha")
        for l in range(LA):
            nc.sync.dma_start(
                out=ha[:, l * D : (l + 1) * D],
                in_=hs[l, 1 + p0 : 1 + p0 + rows, :],
            )
        hb = hst_pool.tile([rows, LB * D], f32, tag="hb", name="hb")
        for l in range(LB):
            nc.sync.dma_start(
                out=hb[:, l * D : (l + 1) * D],
                in_=hs[LA + l, 1 + p0 : 1 + p0 + rows, :],
            )
        a1 = scr_pool.tile([rows, 2 * D], f32, tag="a1")
        nc.vector.tensor_add(a1[:], ha[:, 0 : 2 * D], ha[:, 2 * D : 4 * D])
        nc.gpsimd.tensor_add(ha[:, 0 : 2 * D], hb[:, 0 : 2 * D], hb[:, 2 * D : 4 * D])
        nc.vector.tensor_add(hb[:, 0:D], a1[:, 0:D], a1[:, D : 2 * D])
        nc.gpsimd.tensor_add(hb[:, D : 2 * D], ha[:, 0:D], ha[:, D : 2 * D])
        nc.vector.tensor_add(a1[:, 0:D], hb[:, 0:D], hb[:, D : 2 * D])
        ssum = scr_pool.tile([rows, D], f32, tag="ssum")
        nc.vector.tensor_add(ssum[:], a1[:, 0:D], hb[:, 4 * D : 5 * D])

        # l2 normalize rows
        sumsq = scr_pool.tile([rows, 1], f32, tag="sumsq")
        nc.scalar.activation(
            out=a1[:, D : 2 * D], in_=ssum[:], func=Act.Square, accum_out=sumsq[:]
        )
        nrmv = scr_pool.tile([rows, 1], f32, tag="nrmv")
        nc.scalar.activation(out=nrmv[:], in_=sumsq[:], func=Act.Sqrt)
        nc.vector.tensor_scalar_max(nrmv[:], nrmv[:], 1e-12)
        inv = scr_pool.tile([rows, 1], f32, tag="inv")
        nc.vector.reciprocal(out=inv[:], in_=nrmv[:])
        nrmt = persist.tile([rows, D], f32, tag=f"nrm{ti}", name=f"nrm{ti}")
        nc.vector.tensor_scalar(
            out=nrmt[:], in0=ssum[:], scalar1=inv[:], scalar2=None, op0=Alu.mult
        )
        nrm_tiles.append(nrmt)

        # transpose into nrmT d-chunks; accumulate bnrmT incrementally
        for dt in range(ND):
            psT = ps_t.tile([128, rows], f32, tag="ps_t")
            nc.tensor.transpose(
                psT[:], nrmt[:, 128 * dt : 128 * (dt + 1)], ident[0:rows, 0:rows]
            )
            nc.scalar.copy(out=nrmT[dt][:, p0 : p0 + rows], in_=psT[:])
            psB = ps_bn.tile([128, K], f32, tag="ps_bn")
            nc.tensor.matmul(
                psB[:], nrmt[:, 128 * dt : 128 * (dt + 1)], o_tiles[ti][:],
                start=True, stop=True,
            )
            if ti == 0:
                nc.vector.tensor_copy(out=bnrmT[dt][:], in_=psB[:])
            else:
                nc.vector.tensor_add(bnrmT[dt][:], bnrmT[dt][:], psB[:])

    # agg tiles last: their DMA overlaps the PE tail
    agg_tiles = []
    for ti, (p0, rows) in enumerate(PT):
        aggt = persist.tile([rows, D], f32, tag=f"agg{ti}", name=f"agg{ti}")
        nc.sync.dma_start(out=aggt[:], in_=hagg[1 + p0 : 1 + p0 + rows, :])
        agg_tiles.append(aggt)

    # ---- sim = bench_nrm @ nrm^T  [42, 576] ----
    sim1 = ps_acc.tile([K, 512], f32, tag="acc")
    sim2 = ps_acc.tile([K, 64], f32, tag="acc")
    for dt in range(ND):
        nc.tensor.matmul(
            sim1[:], bnrmT[dt][:], nrmT[dt][:, 0:512],
            start=(dt == 0), stop=(dt == ND - 1),
        )
        nc.tensor.matmul(
            sim2[:], bnrmT[dt][:], nrmT[dt][:, 512:P],
            start=(dt == 0), stop=(dt == ND - 1),
        )

    # ---- w = relu(sim) * dpen; normalize; self weight 1.0 ----
    w = sel.tile([K, P], f32, tag="w")
    nc.scalar.activation(out=w[:, 0:512], in_=sim1[:], func=Act.Relu)
    nc.scalar.activation(out=w[:, 512:P], in_=sim2[:], func=Act.Relu)
    nc.vector.tensor_tensor(out=w[:], in0=w[:], in1=dpen[:], op=Alu.mult)
    wsum = sel.tile([K, 1], f32, tag="wsum")
    nc.vector.tensor_reduce(out=wsum[:], in_=w[:], axis=AX.X, op=Alu.add)
    nc.vector.tensor_scalar_add(wsum[:], wsum[:], 1e-8)
    winv = sel.tile([K, 1], f32, tag="winv")
    nc.vector.reciprocal(out=winv[:], in_=wsum[:])
    nc.vector.tensor_scalar(
        out=w[:], in0=w[:], scalar1=winv[:], scalar2=None, op0=Alu.mult
    )
    nc.vector.copy_predicated(
        out=w[:], mask=oself[:], data=ones_c[0:K, 0:1].to_broadcast([K, P])
    )

    # ---- out = w @ patch_agg  [42, 1024] ----
    wT = []
    for ti, (p0, rows) in enumerate(PT):
        psW = ps_t.tile([128, K], f32, tag="ps_t")
        nc.tensor.transpose(psW[0:rows, 0:K], w[:, p0 : p0 + rows], ident[0:K, 0:K])
        wt = sel.tile([rows, K], f32, tag=f"wT{ti}", name=f"wT{ti}")
        nc.scalar.copy(out=wt[:], in_=psW[0:rows, 0:K])
        wT.append(wt)
    o1 = ps_acc.tile([K, 512], f32, tag="acc")
    o2 = ps_acc.tile([K, 512], f32, tag="acc")
    for ti, (p0, rows) in enumerate(PT):
        nc.tensor.matmul(
            o1[:], wT[ti][:], agg_tiles[ti][:, 0:512],
            start=(ti == 0), stop=(ti == NT - 1),
        )
        nc.tensor.matmul(
            o2[:], wT[ti][:], agg_tiles[ti][:, 512:D],
            start=(ti == 0), stop=(ti == NT - 1),
        )
    outsb = sel.tile([K, D], f32, tag="outsb")
    nc.vector.tensor_copy(out=outsb[:, 0:512], in_=o1[:])
    nc.scalar.copy(out=outsb[:, 512:D], in_=o2[:])
    nc.sync.dma_start(out=out_d, in_=outsb[:])


def build():
    from contextlib import ExitStack

    import concourse.bacc as bacc
    from concourse.tile import TileContext

    nc = bacc.Bacc("TRN2")
    with TileContext(nc) as tc:
        with ExitStack() as ctx:
            _emit(nc, tc, ctx)
    nc.compile()
    return nc


_NC_CACHE = {}


def kernel(attn, hidden_agg, stacked_hs):
    import numpy as np

    from concourse.bass_utils import run_bass_kernel_spmd

    if "nc" not in _NC_CACHE:
        _NC_CACHE["nc"] = build()
    nc = _NC_CACHE["nc"]

    dist = _dist_penalty_np()
    in_maps = [
        {
            "attn": np.ascontiguousarray(attn[b]),
            "hidden_agg": np.ascontiguousarray(hidden_agg[b]),
            "stacked_hs": np.ascontiguousarray(stacked_hs[:, b]),
            "dist": dist,
        }
        for b in range(NCORES)
    ]
    res = run_bass_kernel_spmd(nc, in_maps, list(range(NCORES)))
    out = np.stack([res.results[b]["out"] for b in range(NCORES)]).astype(np.float32)
    bench = np.stack(
        [res.results[b]["bench"].reshape(K) for b in range(NCORES)]
    ).astype(np.int32)
    return out, bench


def profile(inputs, tmpdir=None):
    """Run once under NTFF capture; returns HW exec time in ns (or None).

    Leaves the ntff/pftrace artifacts in ``tmpdir`` for trace analysis.
    """
    import glob as _glob
    import os as _os
    import tempfile

    import numpy as np

    from concourse import bass2jax

    try:
        from trn_agent_boot.trn_boot import _ntff_profile_via_ctypes
    except ImportError:
        return None
    hook = _ntff_profile_via_ctypes("/opt/axon/libaxon_pjrt.so")
    if hook is None:
        return None

    if "nc" not in _NC_CACHE:
        _NC_CACHE["nc"] = build()
    nc = _NC_CACHE["nc"]
    dist = _dist_penalty_np()
    in_maps = [
        {
            "attn": np.ascontiguousarray(inputs["attn"][b]),
            "hidden_agg": np.ascontiguousarray(inputs["hidden_agg"][b]),
            "stacked_hs": np.ascontiguousarray(inputs["stacked_hs"][:, b]),
            "dist": dist,
        }
        for b in range(NCORES)
    ]
    tmpdir = tmpdir or tempfile.mkdtemp(prefix="ntffprof_")
    with hook(tmpdir, [0]):
        bass2jax.run_bass_via_pjrt(nc, in_maps, n_cores=NCORES)
    ntffs = _glob.glob(_os.path.join(tmpdir, "*_body*.ntff"))
    print(f"profile dir: {tmpdir} ({len(ntffs)} ntff)")
    if not ntffs:
        return None

    import gauge.profiler
    from concourse._compat import FishPath

    prof = gauge.profiler.Profile(
        profile_path=FishPath(tmpdir),
        kernel_dev_mode=True,
        profile_on_exit=False,
        bass_kernel=nc.m,
        offline_processing=True,
        fname="*_body*",
    )
    try:
        res = prof.to_perfetto(model_index=(0,))
        if res:
            print("trace:", res[0].trace_path)
            return res[0].exec_time_ns
    except Exception as e:
        print(f"to_perfetto failed: {type(e).__name__}: {e}")
    return None


# revision 14
# speedup vs baseline: 1.2579x; 1.0286x over previous
"""Trainium2 Bass kernel for nn_CLIPVisionTower_Nuwa_abli (topk_masking).

Per-image pipeline (pure batch data-parallel, 1 image per NeuronCore):
  metric  = sum over heads of attn[:, 0, 1:]                  [576]
  mask out each 2x2-region argmin, take global top-42 of the
  remainder, bench = those patch indices ascending             [42]
  nrm     = l2-normalized mean over 9 layers of hs[:, 1:, :]   [576,1024]
  sim     = nrm[bench] @ nrm.T                                 [42,576]
  w       = relu(sim) * dist_penalty[bench]; row-normalize; self weight 1
  out     = w @ hidden_agg[1:, :]                              [42,1024]
"""

import math

import numpy as np

B = 8
HGRID = 24
P = 576  # patches
K = 42  # bench tokens
HEADS = 16
D = 1024
L = 9  # layers
NCORES = 8
PT = [(0, 128), (128, 128), (256, 128), (384, 128), (512, 64)]  # patch tiles
ND = D // 128  # 8 d-chunks of 128
NEG_BIG = -1.0e30
KEY_BIG = 1.0e9


def _dist_penalty_np() -> np.ndarray:
    ys, xs = np.meshgrid(
        np.arange(HGRID, dtype=np.float32),
        np.arange(HGRID, dtype=np.float32),
        indexing="ij",
    )
    coords = np.stack([ys, xs], axis=-1).reshape(-1, 2)
    diff = coords[:, None, :] - coords[None, :, :]
    dist = np.sqrt((diff * diff).sum(axis=-1), dtype=np.float32)
    thresh = np.float32(math.sqrt(280.0))
    return (np.float32(1.0) - np.minimum(dist / thresh, np.float32(1.0))).astype(
        np.float32
    )


def _emit(nc, tc, ctx):
    import concourse.bass as bass
    import concourse.mybir as mybir
    from concourse.masks import make_identity

    f32 = mybir.dt.float32
    i32 = mybir.dt.int32
    Alu = mybir.AluOpType
    Act = mybir.ActivationFunctionType
    AX = mybir.AxisListType
    NT = len(PT)

    attn = nc.dram_tensor("attn", [HEADS, P + 1, P + 1], f32, kind="ExternalInput").ap()
    hagg = nc.dram_tensor("hidden_agg", [P + 1, D], f32, kind="ExternalInput").ap()
    hs = nc.dram_tensor("stacked_hs", [L, P + 1, D], f32, kind="ExternalInput").ap()
    dist = nc.dram_tensor("dist", [P, P], f32, kind="ExternalInput").ap()
    out_d = nc.dram_tensor("out", [K, D], f32, kind="ExternalOutput").ap()
    bench_d = nc.dram_tensor("bench", [1, K], i32, kind="ExternalOutput").ap()

    consts = ctx.enter_context(tc.tile_pool(name="consts", bufs=1))
    sel = ctx.enter_context(tc.tile_pool(name="sel", bufs=1))
    persist = ctx.enter_context(tc.tile_pool(name="persist", bufs=1))
    hst_pool = ctx.enter_context(tc.tile_pool(name="hst", bufs=10))
    scr_pool = ctx.enter_context(tc.tile_pool(name="scr", bufs=2))
    ps_t = ctx.enter_context(tc.tile_pool(name="ps_t", bufs=2, space="PSUM"))
    ps_acc = ctx.enter_context(tc.tile_pool(name="ps_acc", bufs=2, space="PSUM"))
    ps_bn = ctx.enter_context(tc.tile_pool(name="ps_bn", bufs=2, space="PSUM"))

    # ---- constants ----
    ident = consts.tile([128, 128], f32, tag="ident")
    make_identity(nc, ident[:])
    ones16 = consts.tile([16, 1], f32, tag="ones16")
    nc.gpsimd.memset(ones16[:], 1.0)
    ones_c = consts.tile([128, 1], f32, tag="ones_c")
    nc.gpsimd.memset(ones_c[:], 1.0)
    # kidx[p, k] = k + 1 (selection-rank match target)
    kidx_i = consts.tile([128, K], i32, tag="kidx_i")
    nc.gpsimd.iota(kidx_i[:], pattern=[[1, K]], base=1, channel_multiplier=0)
    kidx_f = consts.tile([128, K], f32, tag="kidx_f")
    nc.vector.tensor_copy(out=kidx_f[:], in_=kidx_i[:])
    # iota_p[*, p] = p  (patch index along free dim, replicated per partition)
    iota_pi = consts.tile([128, P], i32, tag="iota_pi")
    nc.gpsimd.iota(iota_pi[:], pattern=[[1, P]], base=0, channel_multiplier=0)
    iota_pf = consts.tile([128, P], f32, tag="iota_pf")
    nc.vector.tensor_copy(out=iota_pf[:], in_=iota_pi[:])
    # pgidx[p, t] = 128*t + p (global patch index per tile column)
    pgidx_i = consts.tile([128, NT], i32, tag="pgidx_i")
    nc.gpsimd.iota(pgidx_i[:], pattern=[[128, NT]], base=0, channel_multiplier=1)
    pgidx_f = consts.tile([128, NT], f32, tag="pgidx_f")
    nc.vector.tensor_copy(out=pgidx_f[:], in_=pgidx_i[:])

    # ---- selection: metric = sum_h attn[h, 0, 1:] ----
    cls = sel.tile([HEADS, P], f32, tag="cls")
    nc.sync.dma_start(out=cls[:], in_=attn[:, 0, 1:])
    met_ps1 = ps_acc.tile([1, 512], f32, tag="acc")
    met_ps2 = ps_acc.tile([1, 64], f32, tag="acc")
    nc.tensor.matmul(met_ps1[:], ones16[:], cls[:, 0:512], start=True, stop=True)
    nc.tensor.matmul(met_ps2[:], ones16[:], cls[:, 512:P], start=True, stop=True)
    metric = sel.tile([1, P], f32, tag="metric")
    nc.vector.tensor_copy(out=metric[0:1, 0:512], in_=met_ps1[:])
    nc.vector.tensor_copy(out=metric[0:1, 512:P], in_=met_ps2[:])

    # ---- region (2x2) argmin -> masked metric ----
    # patch p = (2*ry+dy)*24 + 2*rx+dx  ->  [ry(12), dy(2), rx(12), dx(2)]
    met_r = metric[:].rearrange("p (ry dy rx dx) -> p ry dy rx dx", dy=2, rx=12, dx=2)
    rmin = sel.tile([1, 144], f32, tag="rmin")
    rmin2 = sel.tile([1, 144], f32, tag="rmin2")
    rmin_v = rmin[:].rearrange("p (a b) -> p a b", b=12)
    rmin2_v = rmin2[:].rearrange("p (a b) -> p a b", b=12)
    nc.vector.tensor_tensor(
        out=rmin_v, in0=met_r[:, :, 0, :, 0], in1=met_r[:, :, 0, :, 1], op=Alu.min
    )
    nc.vector.tensor_tensor(
        out=rmin2_v, in0=met_r[:, :, 1, :, 0], in1=met_r[:, :, 1, :, 1], op=Alu.min
    )
    nc.vector.tensor_tensor(out=rmin_v, in0=rmin_v, in1=rmin2_v, op=Alu.min)

    masked = sel.tile([1, P], f32, tag="masked")
    msk_r = masked[:].rearrange("p (ry dy rx dx) -> p ry dy rx dx", dy=2, rx=12, dx=2)
    eq = sel.tile([1, 144], f32, tag="eq")
    eq_v = eq[:].rearrange("p (a b) -> p a b", b=12)
    for dy in range(2):
        for dx in range(2):
            src = met_r[:, :, dy, :, dx]
            nc.vector.tensor_tensor(out=eq_v, in0=src, in1=rmin_v, op=Alu.is_equal)
            # masked = metric + eq * NEG_BIG
            nc.vector.scalar_tensor_tensor(
                out=msk_r[:, :, dy, :, dx],
                in0=eq_v,
                scalar=NEG_BIG,
                in1=src,
                op0=Alu.mult,
                op1=Alu.add,
            )

    # ---- global top-42 threshold (6 rounds of max8 / match_replace) ----
    scratch = sel.tile([1, P], f32, tag="scratch")
    nc.vector.tensor_copy(out=scratch[:], in_=masked[:])
    val48 = sel.tile([1, 48], f32, tag="val48")
    for r in range(6):
        nc.vector.max(out=val48[0:1, 8 * r : 8 * (r + 1)], in_=scratch[:])
        nc.vector.match_replace(
            out=scratch[:],
            in_to_replace=val48[0:1, 8 * r : 8 * (r + 1)],
            in_values=scratch[:],
            imm_value=NEG_BIG,
        )
    # mask of selected patches: masked >= v41 (42nd largest)
    selmask = sel.tile([1, P], f32, tag="selmask")
    nc.vector.tensor_scalar(
        out=selmask[:], in0=masked[:], scalar1=val48[0:1, 41:42], scalar2=None,
        op0=Alu.is_ge,
    )
    # rank of each selected patch: pos = inclusive cumsum(selmask); selected
    # patch with rank r has poskey = r+1, unselected 0.
    pos = sel.tile([1, P], f32, tag="pos")
    nc.vector.tensor_tensor_scan(
        out=pos[:], data0=selmask[:], data1=selmask[:], initial=0.0,
        op0=Alu.add, op1=Alu.bypass,
    )
    poskey = sel.tile([1, P], f32, tag="poskey")
    nc.vector.tensor_tensor(out=poskey[:], in0=pos[:], in1=selmask[:], op=Alu.mult)

    # redistribute poskey to partitions (PE transpose per patch tile), then
    # one-hot O[p, k] = (poskey[p] == k+1)
    o_tiles = []
    for ti, (p0, rows) in enumerate(PT):
        psP = ps_bn.tile([128, 1], f32, tag="ps_bn")
        nc.tensor.transpose(
            psP[0:rows, 0:1], poskey[0:1, p0 : p0 + rows], ident[0:1, 0:1]
        )
        pcol = sel.tile([rows, 1], f32, tag=f"pcol{ti}", name=f"pcol{ti}")
        nc.scalar.copy(out=pcol[:], in_=psP[0:rows, 0:1])
        ot = sel.tile([rows, K], f32, tag=f"oh{ti}", name=f"oh{ti}")
        nc.vector.tensor_scalar(
            out=ot[:], in0=kidx_f[0:rows, :], scalar1=pcol[:], scalar2=None,
            op0=Alu.is_equal,
        )
        o_tiles.append(ot)

    # bench values: bench[k] = sum_p p * O[p, k]  (exact small ints in f32)
    psV = ps_bn.tile([1, K], f32, tag="ps_bn")
    for ti, (p0, rows) in enumerate(PT):
        nc.tensor.matmul(
            psV[:], pgidx_f[0:rows, ti : ti + 1], o_tiles[ti][:],
            start=(ti == 0), stop=(ti == NT - 1),
        )
    bench_i = sel.tile([1, K], i32, tag="bench_i")
    nc.vector.tensor_copy(out=bench_i[:], in_=psV[:])
    nc.sync.dma_start(out=bench_d, in_=bench_i[:])

    # bench as an int32 column (one index per partition) for indirect gathers
    benchfr = sel.tile([1, K], f32, tag="benchfr")
    nc.scalar.copy(out=benchfr[:], in_=psV[:])
    psC = ps_bn.tile([128, 1], f32, tag="ps_bn")
    nc.tensor.transpose(psC[0:K, 0:1], benchfr[:], ident[0:1, 0:1])
    benchcol = sel.tile([K, 1], i32, tag="benchcol")
    nc.vector.tensor_copy(out=benchcol[:], in_=psC[0:K, 0:1])

    # dpen[k, :] = dist_penalty[bench[k], :] via indirect row gather
    dpen = sel.tile([K, P], f32, tag="dpen")
    nc.gpsimd.indirect_dma_start(
        out=dpen[:],
        out_offset=None,
        in_=dist,
        in_offset=bass.IndirectOffsetOnAxis(ap=benchcol[:, 0:1], axis=0),
    )

    # oself[k, p] = (p == bench[k]) int mask for copy_predicated
    benchcf = sel.tile([K, 1], f32, tag="benchcf")
    nc.scalar.copy(out=benchcf[:], in_=psC[0:K, 0:1])
    oself = sel.tile([K, P], i32, tag="oself")
    nc.vector.tensor_scalar(
        out=oself[:], in0=iota_pf[0:K, :], scalar1=benchcf[:], scalar2=None,
        op0=Alu.is_equal,
    )

    # ---- heavy phase: layer sum (scale dropped; nrm is scale-invariant) ----
    nrm_tiles = []
    nrmT = [
        persist.tile([128, P], f32, tag=f"nrmT{dt}", name=f"nrmT{dt}")
        for dt in range(ND)
    ]
    bnrmT = [
        sel.tile([128, K], f32, tag=f"bnrmT{dt}", name=f"bnrmT{dt}")
        for dt in range(ND)
    ]
    LA, LB = 4, 5
    for ti, (p0, rows) in enumerate(PT):
        # stream 9 layers into per-layer slots (freed right after their add);
        # DMA issue split across the two HWDGE queues (sync + scalar)
        lt = []
        for l in range(L):
            t = hst_pool.tile([rows, D], f32, tag="hst", name="hst")
            dmae = nc.sync if l % 2 == 0 else nc.scalar
            dmae.dma_start(out=t[:], in_=hs[l, 1 + p0 : 1 + p0 + rows, :])
            lt.append(t)
        accV = scr_pool.tile([rows, D], f32, tag="accV")
        nc.vector.tensor_add(accV[:], lt[0][:], lt[1][:])
        nc.vector.tensor_add(accV[:], accV[:], lt[2][:])
        nc.vector.tensor_add(accV[:], accV[:], lt[3][:])
        nc.vector.tensor_add(accV[:], accV[:], lt[4][:])
        nc.vector.tensor_add(accV[:], accV[:], lt[5][:])
        accG = scr_pool.tile([rows, D], f32, tag="accG")
        nc.gpsimd.tensor_add(accG[:], lt[6][:], lt[7][:])
        nc.gpsimd.tensor_add(accG[:], accG[:], lt[8][:])
        ssum = scr_pool.tile([rows, D], f32, tag="ssum")
        nc.vector.tensor_add(ssum[:], accV[:], accG[:])

        # l2 normalize rows
        sumsq = scr_pool.tile([rows, 1], f32, tag="sumsq")
        sq = scr_pool.tile([rows, D], f32, tag="sq", bufs=1)
        nc.scalar.activation(
            out=sq[:], in_=ssum[:], func=Act.Square, accum_out=sumsq[:]
        )
        nrmv = scr_pool.tile([rows, 1], f32, tag="nrmv")
        nc.scalar.activation(out=nrmv[:], in_=sumsq[:], func=Act.Sqrt)
        nc.vector.tensor_scalar_max(nrmv[:], nrmv[:], 1e-12)
        inv = scr_pool.tile([rows, 1], f32, tag="inv")
        nc.vector.reciprocal(out=inv[:], in_=nrmv[:])
        nrmt = persist.tile([rows, D], f32, tag=f"nrm{ti}", name=f"nrm{ti}")
        nc.vector.tensor_scalar(
            out=nrmt[:], in0=ssum[:], scalar1=inv[:], scalar2=None, op0=Alu.mult
        )
        nrm_tiles.append(nrmt)

        # transpose into nrmT d-chunks; accumulate bnrmT incrementally
        for dt in range(ND):
            psT = ps_t.tile([128, rows], f32, tag="ps_t")
            nc.tensor.transpose(
                psT[:], nrmt[:, 128 * dt : 128 * (dt + 1)], ident[0:rows, 0:rows]
            )
            nc.scalar.copy(out=nrmT[dt][:, p0 : p0 + rows], in_=psT[:])
            psB = ps_bn.tile([128, K], f32, tag="ps_bn")
            nc.tensor.matmul(
                psB[:], nrmt[:, 128 * dt : 128 * (dt + 1)], o_tiles[ti][:],
                start=True, stop=True,
            )
            if ti == 0:
                nc.vector.tensor_copy(out=bnrmT[dt][:], in_=psB[:])
            else:
                nc.vector.tensor_add(bnrmT[dt][:], bnrmT[dt][:], psB[:])

    # agg tiles last: their DMA overlaps the PE tail
    agg_tiles = []
    for ti, (p0, rows) in enumerate(PT):
        aggt = persist.tile([rows, D], f32, tag=f"agg{ti}", name=f"agg{ti}")
        nc.sync.dma_start(out=aggt[:], in_=hagg[1 + p0 : 1 + p0 + rows, :])
        agg_tiles.append(aggt)

    # ---- sim = bench_nrm @ nrm^T  [42, 576] ----
    sim1 = ps_acc.tile([K, 512], f32, tag="acc")
    sim2 = ps_acc.tile([K, 64], f32, tag="acc")
    for dt in range(ND):
        nc.tensor.matmul(
            sim1[:], bnrmT[dt][:], nrmT[dt][:, 0:512],
            start=(dt == 0), stop=(dt == ND - 1),
        )
        nc.tensor.matmul(
            sim2[:], bnrmT[dt][:], nrmT[dt][:, 512:P],
            start=(dt == 0), stop=(dt == ND - 1),
        )

    # ---- w = relu(sim) * dpen; normalize; self weight 1.0 ----
    w = sel.tile([K, P], f32, tag="w")
    nc.scalar.activation(out=w[:, 0:512], in_=sim1[:], func=Act.Relu)
    nc.scalar.activation(out=w[:, 512:P], in_=sim2[:], func=Act.Relu)
    nc.vector.tensor_tensor(out=w[:], in0=w[:], in1=dpen[:], op=Alu.mult)
    wsum = sel.tile([K, 1], f32, tag="wsum")
    nc.vector.tensor_reduce(out=wsum[:], in_=w[:], axis=AX.X, op=Alu.add)
    nc.vector.tensor_scalar_add(wsum[:], wsum[:], 1e-8)
    winv = sel.tile([K, 1], f32, tag="winv")
    nc.vector.reciprocal(out=winv[:], in_=wsum[:])
    nc.vector.tensor_scalar(
        out=w[:], in0=w[:], scalar1=winv[:], scalar2=None, op0=Alu.mult
    )
    nc.vector.copy_predicated(
        out=w[:], mask=oself[:], data=ones_c[0:K, 0:1].to_broadcast([K, P])
    )

    # ---- out = w @ patch_agg  [42, 1024] ----
    wT = []
    for ti, (p0, rows) in enumerate(PT):
        psW = ps_t.tile([128, K], f32, tag="ps_t")
        nc.tensor.transpose(psW[0:rows, 0:K], w[:, p0 : p0 + rows], ident[0:K, 0:K])
        wt = sel.tile([rows, K], f32, tag=f"wT{ti}", name=f"wT{ti}")
        nc.scalar.copy(out=wt[:], in_=psW[0:rows, 0:K])
        wT.append(wt)
    o1 = ps_acc.tile([K, 512], f32, tag="acc")
    o2 = ps_acc.tile([K, 512], f32, tag="acc")
    for ti, (p0, rows) in enumerate(PT):
        nc.tensor.matmul(
            o1[:], wT[ti][:], agg_tiles[ti][:, 0:512],
            start=(ti == 0), stop=(ti == NT - 1),
        )
        nc.tensor.matmul(
            o2[:], wT[ti][:], agg_tiles[ti][:, 512:D],
            start=(ti == 0), stop=(ti == NT - 1),
        )
    outsb = sel.tile([K, D], f32, tag="outsb")
    nc.vector.tensor_copy(out=outsb[:, 0:512], in_=o1[:])
    nc.scalar.copy(out=outsb[:, 512:D], in_=o2[:])
    nc.sync.dma_start(out=out_d, in_=outsb[:])


def build():
    from contextlib import ExitStack

    import concourse.bacc as bacc
    from concourse.tile import TileContext

    nc = bacc.Bacc("TRN2")
    with TileContext(nc) as tc:
        with ExitStack() as ctx:
            _emit(nc, tc, ctx)
    nc.compile()
    return nc


_NC_CACHE = {}


def kernel(attn, hidden_agg, stacked_hs):
    import numpy as np

    from concourse.bass_utils import run_bass_kernel_spmd

    if "nc" not in _NC_CACHE:
        _NC_CACHE["nc"] = build()
    nc = _NC_CACHE["nc"]

    dist = _dist_penalty_np()
    in_maps = [
        {
            "attn": np.ascontiguousarray(attn[b]),
            "hidden_agg": np.ascontiguousarray(hidden_agg[b]),
            "stacked_hs": np.ascontiguousarray(stacked_hs[:, b]),
            "dist": dist,
        }
        for b in range(NCORES)
    ]
    res = run_bass_kernel_spmd(nc, in_maps, list(range(NCORES)))
    out = np.stack([res.results[b]["out"] for b in range(NCORES)]).astype(np.float32)
    bench = np.stack(
        [res.results[b]["bench"].reshape(K) for b in range(NCORES)]
    ).astype(np.int32)
    return out, bench


def profile(inputs, tmpdir=None):
    """Run once under NTFF capture; returns HW exec time in ns (or None).

    Leaves the ntff/pftrace artifacts in ``tmpdir`` for trace analysis.
    """
    import glob as _glob
    import os as _os
    import tempfile

    import numpy as np

    from concourse import bass2jax

    try:
        from trn_agent_boot.trn_boot import _ntff_profile_via_ctypes
    except ImportError:
        return None
    hook = _ntff_profile_via_ctypes("/opt/axon/libaxon_pjrt.so")
    if hook is None:
        return None

    if "nc" not in _NC_CACHE:
        _NC_CACHE["nc"] = build()
    nc = _NC_CACHE["nc"]
    dist = _dist_penalty_np()
    in_maps = [
        {
            "attn": np.ascontiguousarray(inputs["attn"][b]),
            "hidden_agg": np.ascontiguousarray(inputs["hidden_agg"][b]),
            "stacked_hs": np.ascontiguousarray(inputs["stacked_hs"][:, b]),
            "dist": dist,
        }
        for b in range(NCORES)
    ]
    tmpdir = tmpdir or tempfile.mkdtemp(prefix="ntffprof_")
    with hook(tmpdir, [0]):
        bass2jax.run_bass_via_pjrt(nc, in_maps, n_cores=NCORES)
    ntffs = _glob.glob(_os.path.join(tmpdir, "*_body*.ntff"))
    print(f"profile dir: {tmpdir} ({len(ntffs)} ntff)")
    if not ntffs:
        return None

    import gauge.profiler
    from concourse._compat import FishPath

    prof = gauge.profiler.Profile(
        profile_path=FishPath(tmpdir),
        kernel_dev_mode=True,
        profile_on_exit=False,
        bass_kernel=nc.m,
        offline_processing=True,
        fname="*_body*",
    )
    try:
        res = prof.to_perfetto(model_index=(0,))
        if res:
            print("trace:", res[0].trace_path)
            return res[0].exec_time_ns
    except Exception as e:
        print(f"to_perfetto failed: {type(e).__name__}: {e}")
    return None


# revision 15
# speedup vs baseline: 1.3122x; 1.0432x over previous
"""Trainium2 Bass kernel for nn_CLIPVisionTower_Nuwa_abli (topk_masking).

Per-image pipeline (pure batch data-parallel, 1 image per NeuronCore):
  metric  = sum over heads of attn[:, 0, 1:]                  [576]
  mask out each 2x2-region argmin, take global top-42 of the
  remainder, bench = those patch indices ascending             [42]
  nrm     = l2-normalized mean over 9 layers of hs[:, 1:, :]   [576,1024]
  sim     = nrm[bench] @ nrm.T                                 [42,576]
  w       = relu(sim) * dist_penalty[bench]; row-normalize; self weight 1
  out     = w @ hidden_agg[1:, :]                              [42,1024]
"""

import math

import numpy as np

B = 8
HGRID = 24
P = 576  # patches
K = 42  # bench tokens
HEADS = 16
D = 1024
L = 9  # layers
NCORES = 8
PT = [(0, 128), (128, 128), (256, 128), (384, 128), (512, 64)]  # patch tiles
ND = D // 128  # 8 d-chunks of 128
NEG_BIG = -1.0e30
KEY_BIG = 1.0e9


def _dist_penalty_np() -> np.ndarray:
    ys, xs = np.meshgrid(
        np.arange(HGRID, dtype=np.float32),
        np.arange(HGRID, dtype=np.float32),
        indexing="ij",
    )
    coords = np.stack([ys, xs], axis=-1).reshape(-1, 2)
    diff = coords[:, None, :] - coords[None, :, :]
    dist = np.sqrt((diff * diff).sum(axis=-1), dtype=np.float32)
    thresh = np.float32(math.sqrt(280.0))
    return (np.float32(1.0) - np.minimum(dist / thresh, np.float32(1.0))).astype(
        np.float32
    )


def _emit(nc, tc, ctx):
    import concourse.bass as bass
    import concourse.mybir as mybir
    from concourse.masks import make_identity

    f32 = mybir.dt.float32
    i32 = mybir.dt.int32
    Alu = mybir.AluOpType
    Act = mybir.ActivationFunctionType
    AX = mybir.AxisListType
    NT = len(PT)

    attn = nc.dram_tensor("attn", [HEADS, P + 1, P + 1], f32, kind="ExternalInput").ap()
    hagg = nc.dram_tensor("hidden_agg", [P + 1, D], f32, kind="ExternalInput").ap()
    hs = nc.dram_tensor("stacked_hs", [L, P + 1, D], f32, kind="ExternalInput").ap()
    dist = nc.dram_tensor("dist", [P, P], f32, kind="ExternalInput").ap()
    out_d = nc.dram_tensor("out", [K, D], f32, kind="ExternalOutput").ap()
    bench_d = nc.dram_tensor("bench", [1, K], i32, kind="ExternalOutput").ap()
    nrm_dram = nc.dram_tensor("nrm_scr", [P, D], f32).ap()

    consts = ctx.enter_context(tc.tile_pool(name="consts", bufs=1))
    sel = ctx.enter_context(tc.tile_pool(name="sel", bufs=1))
    persist = ctx.enter_context(tc.tile_pool(name="persist", bufs=1))
    hst_pool = ctx.enter_context(tc.tile_pool(name="hst", bufs=8))
    scr_pool = ctx.enter_context(tc.tile_pool(name="scr", bufs=3))
    ps_t = ctx.enter_context(tc.tile_pool(name="ps_t", bufs=3, space="PSUM"))
    ps_acc = ctx.enter_context(tc.tile_pool(name="ps_acc", bufs=2, space="PSUM"))
    ps_bn = ctx.enter_context(tc.tile_pool(name="ps_bn", bufs=3, space="PSUM"))

    # ---- constants ----
    ident = consts.tile([128, 128], f32, tag="ident")
    make_identity(nc, ident[:])
    ones16 = consts.tile([16, 1], f32, tag="ones16")
    nc.gpsimd.memset(ones16[:], 1.0)
    ones_c = consts.tile([128, 1], f32, tag="ones_c")
    nc.gpsimd.memset(ones_c[:], 1.0)
    # kidx[p, k] = k + 1 (selection-rank match target)
    kidx_i = consts.tile([128, K], i32, tag="kidx_i")
    nc.gpsimd.iota(kidx_i[:], pattern=[[1, K]], base=1, channel_multiplier=0)
    kidx_f = consts.tile([128, K], f32, tag="kidx_f")
    nc.vector.tensor_copy(out=kidx_f[:], in_=kidx_i[:])
    # iota_p[*, p] = p  (patch index along free dim, replicated per partition)
    iota_pi = consts.tile([128, P], i32, tag="iota_pi")
    nc.gpsimd.iota(iota_pi[:], pattern=[[1, P]], base=0, channel_multiplier=0)
    iota_pf = consts.tile([128, P], f32, tag="iota_pf")
    nc.vector.tensor_copy(out=iota_pf[:], in_=iota_pi[:])
    # pgidx[p, t] = 128*t + p (global patch index per tile column)
    pgidx_i = consts.tile([128, NT], i32, tag="pgidx_i")
    nc.gpsimd.iota(pgidx_i[:], pattern=[[128, NT]], base=0, channel_multiplier=1)
    pgidx_f = consts.tile([128, NT], f32, tag="pgidx_f")
    nc.vector.tensor_copy(out=pgidx_f[:], in_=pgidx_i[:])

    # ---- selection: metric = sum_h attn[h, 0, 1:] ----
    cls = sel.tile([HEADS, P], f32, tag="cls")
    nc.sync.dma_start(out=cls[:], in_=attn[:, 0, 1:])
    met_ps1 = ps_acc.tile([1, 512], f32, tag="acc")
    met_ps2 = ps_acc.tile([1, 64], f32, tag="acc")
    nc.tensor.matmul(met_ps1[:], ones16[:], cls[:, 0:512], start=True, stop=True)
    nc.tensor.matmul(met_ps2[:], ones16[:], cls[:, 512:P], start=True, stop=True)
    metric = sel.tile([1, P], f32, tag="metric")
    nc.vector.tensor_copy(out=metric[0:1, 0:512], in_=met_ps1[:])
    nc.vector.tensor_copy(out=metric[0:1, 512:P], in_=met_ps2[:])

    # ---- region (2x2) argmin -> masked metric ----
    # patch p = (2*ry+dy)*24 + 2*rx+dx  ->  [ry(12), dy(2), rx(12), dx(2)]
    met_r = metric[:].rearrange("p (ry dy rx dx) -> p ry dy rx dx", dy=2, rx=12, dx=2)
    rmin = sel.tile([1, 144], f32, tag="rmin")
    rmin2 = sel.tile([1, 144], f32, tag="rmin2")
    rmin_v = rmin[:].rearrange("p (a b) -> p a b", b=12)
    rmin2_v = rmin2[:].rearrange("p (a b) -> p a b", b=12)
    nc.vector.tensor_tensor(
        out=rmin_v, in0=met_r[:, :, 0, :, 0], in1=met_r[:, :, 0, :, 1], op=Alu.min
    )
    nc.vector.tensor_tensor(
        out=rmin2_v, in0=met_r[:, :, 1, :, 0], in1=met_r[:, :, 1, :, 1], op=Alu.min
    )
    nc.vector.tensor_tensor(out=rmin_v, in0=rmin_v, in1=rmin2_v, op=Alu.min)

    masked = sel.tile([1, P], f32, tag="masked")
    msk_r = masked[:].rearrange("p (ry dy rx dx) -> p ry dy rx dx", dy=2, rx=12, dx=2)
    eq = sel.tile([1, 144], f32, tag="eq")
    eq_v = eq[:].rearrange("p (a b) -> p a b", b=12)
    for dy in range(2):
        for dx in range(2):
            src = met_r[:, :, dy, :, dx]
            nc.vector.tensor_tensor(out=eq_v, in0=src, in1=rmin_v, op=Alu.is_equal)
            # masked = metric + eq * NEG_BIG
            nc.vector.scalar_tensor_tensor(
                out=msk_r[:, :, dy, :, dx],
                in0=eq_v,
                scalar=NEG_BIG,
                in1=src,
                op0=Alu.mult,
                op1=Alu.add,
            )

    # ---- global top-42 threshold (6 rounds of max8 / match_replace) ----
    scratch = sel.tile([1, P], f32, tag="scratch")
    nc.vector.tensor_copy(out=scratch[:], in_=masked[:])
    val48 = sel.tile([1, 48], f32, tag="val48")
    for r in range(6):
        nc.vector.max(out=val48[0:1, 8 * r : 8 * (r + 1)], in_=scratch[:])
        nc.vector.match_replace(
            out=scratch[:],
            in_to_replace=val48[0:1, 8 * r : 8 * (r + 1)],
            in_values=scratch[:],
            imm_value=NEG_BIG,
        )
    # mask of selected patches: masked >= v41 (42nd largest)
    selmask = sel.tile([1, P], f32, tag="selmask")
    nc.vector.tensor_scalar(
        out=selmask[:], in0=masked[:], scalar1=val48[0:1, 41:42], scalar2=None,
        op0=Alu.is_ge,
    )
    # rank of each selected patch: pos = inclusive cumsum(selmask); selected
    # patch with rank r has poskey = r+1, unselected 0.
    pos = sel.tile([1, P], f32, tag="pos")
    nc.vector.tensor_tensor_scan(
        out=pos[:], data0=selmask[:], data1=selmask[:], initial=0.0,
        op0=Alu.add, op1=Alu.bypass,
    )
    poskey = sel.tile([1, P], f32, tag="poskey")
    nc.vector.tensor_tensor(out=poskey[:], in0=pos[:], in1=selmask[:], op=Alu.mult)

    # redistribute poskey to partitions (PE transpose per patch tile), then
    # one-hot O[p, k] = (poskey[p] == k+1)
    o_tiles = []
    for ti, (p0, rows) in enumerate(PT):
        psP = ps_bn.tile([128, 1], f32, tag="ps_bn")
        nc.tensor.transpose(
            psP[0:rows, 0:1], poskey[0:1, p0 : p0 + rows], ident[0:1, 0:1]
        )
        pcol = sel.tile([rows, 1], f32, tag=f"pcol{ti}", name=f"pcol{ti}")
        nc.scalar.copy(out=pcol[:], in_=psP[0:rows, 0:1])
        ot = sel.tile([rows, K], f32, tag=f"oh{ti}", name=f"oh{ti}")
        nc.vector.tensor_scalar(
            out=ot[:], in0=kidx_f[0:rows, :], scalar1=pcol[:], scalar2=None,
            op0=Alu.is_equal,
        )
        o_tiles.append(ot)

    # bench values: bench[k] = sum_p p * O[p, k]  (exact small ints in f32)
    psV = ps_bn.tile([1, K], f32, tag="ps_bn")
    for ti, (p0, rows) in enumerate(PT):
        nc.tensor.matmul(
            psV[:], pgidx_f[0:rows, ti : ti + 1], o_tiles[ti][:],
            start=(ti == 0), stop=(ti == NT - 1),
        )
    bench_i = sel.tile([1, K], i32, tag="bench_i")
    nc.vector.tensor_copy(out=bench_i[:], in_=psV[:])
    nc.sync.dma_start(out=bench_d, in_=bench_i[:])

    # bench as an int32 column (one index per partition) for indirect gathers
    benchfr = sel.tile([1, K], f32, tag="benchfr")
    nc.scalar.copy(out=benchfr[:], in_=psV[:])
    psC = ps_bn.tile([128, 1], f32, tag="ps_bn")
    nc.tensor.transpose(psC[0:K, 0:1], benchfr[:], ident[0:1, 0:1])
    benchcol = sel.tile([K, 1], i32, tag="benchcol")
    nc.vector.tensor_copy(out=benchcol[:], in_=psC[0:K, 0:1])

    # dpen[k, :] = dist_penalty[bench[k], :] via indirect row gather
    dpen = sel.tile([K, P], f32, tag="dpen")
    nc.gpsimd.indirect_dma_start(
        out=dpen[:],
        out_offset=None,
        in_=dist,
        in_offset=bass.IndirectOffsetOnAxis(ap=benchcol[:, 0:1], axis=0),
    )

    # oself[k, p] = (p == bench[k]) int mask for copy_predicated
    benchcf = sel.tile([K, 1], f32, tag="benchcf")
    nc.scalar.copy(out=benchcf[:], in_=psC[0:K, 0:1])
    oself = sel.tile([K, P], i32, tag="oself")
    nc.vector.tensor_scalar(
        out=oself[:], in0=iota_pf[0:K, :], scalar1=benchcf[:], scalar2=None,
        op0=Alu.is_equal,
    )

    # ---- heavy phase: layer sum (scale dropped; nrm is scale-invariant) ----
    nrm_tiles = []
    nrmT = [
        persist.tile([128, P], f32, tag=f"nrmT{dt}", name=f"nrmT{dt}")
        for dt in range(ND)
    ]
    LA, LB = 4, 5
    for ti, (p0, rows) in enumerate(PT):
        # stream 9 layers into per-layer slots (freed right after their add);
        # DMA issue split across the two HWDGE queues (sync + scalar)
        lt = []
        for l in range(L):
            t = hst_pool.tile([rows, D], f32, tag="hst", name="hst")
            dmae = nc.sync if l % 2 == 0 else nc.scalar
            dmae.dma_start(out=t[:], in_=hs[l, 1 + p0 : 1 + p0 + rows, :])
            lt.append(t)
        accV = scr_pool.tile([rows, D], f32, tag="accV")
        nc.vector.tensor_add(accV[:], lt[0][:], lt[1][:])
        nc.vector.tensor_add(accV[:], accV[:], lt[2][:])
        nc.vector.tensor_add(accV[:], accV[:], lt[3][:])
        nc.vector.tensor_add(accV[:], accV[:], lt[4][:])
        nc.vector.tensor_add(accV[:], accV[:], lt[5][:])
        accG = scr_pool.tile([rows, D], f32, tag="accG")
        nc.gpsimd.tensor_add(accG[:], lt[6][:], lt[7][:])
        nc.gpsimd.tensor_add(accG[:], accG[:], lt[8][:])
        ssum = scr_pool.tile([rows, D], f32, tag="ssum")
        nc.vector.tensor_add(ssum[:], accV[:], accG[:])

        # l2 normalize rows
        sumsq = scr_pool.tile([rows, 1], f32, tag="sumsq")
        sq = scr_pool.tile([rows, D], f32, tag="sq", bufs=1)
        nc.scalar.activation(
            out=sq[:], in_=ssum[:], func=Act.Square, accum_out=sumsq[:]
        )
        nrmv = scr_pool.tile([rows, 1], f32, tag="nrmv")
        nc.scalar.activation(out=nrmv[:], in_=sumsq[:], func=Act.Sqrt)
        nc.vector.tensor_scalar_max(nrmv[:], nrmv[:], 1e-12)
        inv = scr_pool.tile([rows, 1], f32, tag="inv")
        nc.vector.reciprocal(out=inv[:], in_=nrmv[:])
        nrmt = persist.tile([rows, D], f32, tag=f"nrm{ti}", name=f"nrm{ti}")
        nc.vector.tensor_scalar(
            out=nrmt[:], in0=ssum[:], scalar1=inv[:], scalar2=None, op0=Alu.mult
        )
        nrm_tiles.append(nrmt)

        # nrm row block to DRAM scratch (for the bench-row gather)
        nc.sync.dma_start(out=nrm_dram[p0 : p0 + rows, :], in_=nrmt[:])

        # transpose into nrmT d-chunks
        for dt in range(ND):
            psT = ps_t.tile([128, rows], f32, tag="ps_t")
            nc.tensor.transpose(
                psT[:], nrmt[:, 128 * dt : 128 * (dt + 1)], ident[0:rows, 0:rows]
            )
            nc.scalar.copy(out=nrmT[dt][:, p0 : p0 + rows], in_=psT[:])

    # agg tiles last: their DMA overlaps the PE tail
    agg_tiles = []
    for ti, (p0, rows) in enumerate(PT):
        aggt = persist.tile([rows, D], f32, tag=f"agg{ti}", name=f"agg{ti}")
        nc.sync.dma_start(out=aggt[:], in_=hagg[1 + p0 : 1 + p0 + rows, :])
        agg_tiles.append(aggt)

    # ---- bench_nrm rows via indirect gather, then transpose to d-major ----
    bnrm = sel.tile([K, D], f32, tag="bnrm")
    nc.gpsimd.indirect_dma_start(
        out=bnrm[:],
        out_offset=None,
        in_=nrm_dram,
        in_offset=bass.IndirectOffsetOnAxis(ap=benchcol[:, 0:1], axis=0),
    )
    bnrmT = []
    for dt in range(ND):
        psBT = ps_bn.tile([128, K], f32, tag="ps_bn")
        nc.tensor.transpose(
            psBT[:], bnrm[:, 128 * dt : 128 * (dt + 1)], ident[0:K, 0:K]
        )
        bt = sel.tile([128, K], f32, tag=f"bnrmT{dt}", name=f"bnrmT{dt}")
        nc.scalar.copy(out=bt[:], in_=psBT[:])
        bnrmT.append(bt)

    # ---- sim = bench_nrm @ nrm^T  [42, 576] ----
    sim1 = ps_acc.tile([K, 512], f32, tag="acc")
    sim2 = ps_acc.tile([K, 64], f32, tag="acc")
    for dt in range(ND):
        nc.tensor.matmul(
            sim1[:], bnrmT[dt][:], nrmT[dt][:, 0:512],
            start=(dt == 0), stop=(dt == ND - 1),
        )
        nc.tensor.matmul(
            sim2[:], bnrmT[dt][:], nrmT[dt][:, 512:P],
            start=(dt == 0), stop=(dt == ND - 1),
        )

    # ---- w = relu(sim) * dpen; normalize; self weight 1.0 ----
    w = sel.tile([K, P], f32, tag="w")
    nc.scalar.activation(out=w[:, 0:512], in_=sim1[:], func=Act.Relu)
    nc.scalar.activation(out=w[:, 512:P], in_=sim2[:], func=Act.Relu)
    nc.vector.tensor_tensor(out=w[:], in0=w[:], in1=dpen[:], op=Alu.mult)
    wsum = sel.tile([K, 1], f32, tag="wsum")
    nc.vector.tensor_reduce(out=wsum[:], in_=w[:], axis=AX.X, op=Alu.add)
    nc.vector.tensor_scalar_add(wsum[:], wsum[:], 1e-8)
    winv = sel.tile([K, 1], f32, tag="winv")
    nc.vector.reciprocal(out=winv[:], in_=wsum[:])
    nc.vector.tensor_scalar(
        out=w[:], in0=w[:], scalar1=winv[:], scalar2=None, op0=Alu.mult
    )
    nc.vector.copy_predicated(
        out=w[:], mask=oself[:], data=ones_c[0:K, 0:1].to_broadcast([K, P])
    )

    # ---- out = w @ patch_agg  [42, 1024] ----
    wT = []
    for ti, (p0, rows) in enumerate(PT):
        psW = ps_t.tile([128, K], f32, tag="ps_t")
        nc.tensor.transpose(psW[0:rows, 0:K], w[:, p0 : p0 + rows], ident[0:K, 0:K])
        wt = sel.tile([rows, K], f32, tag=f"wT{ti}", name=f"wT{ti}")
        nc.scalar.copy(out=wt[:], in_=psW[0:rows, 0:K])
        wT.append(wt)
    o1 = ps_acc.tile([K, 512], f32, tag="acc")
    o2 = ps_acc.tile([K, 512], f32, tag="acc")
    for ti, (p0, rows) in enumerate(PT):
        nc.tensor.matmul(
            o1[:], wT[ti][:], agg_tiles[ti][:, 0:512],
            start=(ti == 0), stop=(ti == NT - 1),
        )
        nc.tensor.matmul(
            o2[:], wT[ti][:], agg_tiles[ti][:, 512:D],
            start=(ti == 0), stop=(ti == NT - 1),
        )
    outsb = sel.tile([K, D], f32, tag="outsb")
    nc.vector.tensor_copy(out=outsb[:, 0:512], in_=o1[:])
    nc.scalar.copy(out=outsb[:, 512:D], in_=o2[:])
    nc.sync.dma_start(out=out_d, in_=outsb[:])


def build():
    from contextlib import ExitStack

    import concourse.bacc as bacc
    from concourse.tile import TileContext

    nc = bacc.Bacc("TRN2")
    with TileContext(nc) as tc:
        with ExitStack() as ctx:
            _emit(nc, tc, ctx)
    nc.compile()
    return nc


_NC_CACHE = {}


def kernel(attn, hidden_agg, stacked_hs):
    import numpy as np

    from concourse.bass_utils import run_bass_kernel_spmd

    if "nc" not in _NC_CACHE:
        _NC_CACHE["nc"] = build()
    nc = _NC_CACHE["nc"]

    dist = _dist_penalty_np()
    in_maps = [
        {
            "attn": np.ascontiguousarray(attn[b]),
            "hidden_agg": np.ascontiguousarray(hidden_agg[b]),
            "stacked_hs": np.ascontiguousarray(stacked_hs[:, b]),
            "dist": dist,
        }
        for b in range(NCORES)
    ]
    res = run_bass_kernel_spmd(nc, in_maps, list(range(NCORES)))
    out = np.stack([res.results[b]["out"] for b in range(NCORES)]).astype(np.float32)
    bench = np.stack(
        [res.results[b]["bench"].reshape(K) for b in range(NCORES)]
    ).astype(np.int32)
    return out, bench


def profile(inputs, tmpdir=None):
    """Run once under NTFF capture; returns HW exec time in ns (or None).

    Leaves the ntff/pftrace artifacts in ``tmpdir`` for trace analysis.
    """
    import glob as _glob
    import os as _os
    import tempfile

    import numpy as np

    from concourse import bass2jax

    try:
        from trn_agent_boot.trn_boot import _ntff_profile_via_ctypes
    except ImportError:
        return None
    hook = _ntff_profile_via_ctypes("/opt/axon/libaxon_pjrt.so")
    if hook is None:
        return None

    if "nc" not in _NC_CACHE:
        _NC_CACHE["nc"] = build()
    nc = _NC_CACHE["nc"]
    dist = _dist_penalty_np()
    in_maps = [
        {
            "attn": np.ascontiguousarray(inputs["attn"][b]),
            "hidden_agg": np.ascontiguousarray(inputs["hidden_agg"][b]),
            "stacked_hs": np.ascontiguousarray(inputs["stacked_hs"][:, b]),
            "dist": dist,
        }
        for b in range(NCORES)
    ]
    tmpdir = tmpdir or tempfile.mkdtemp(prefix="ntffprof_")
    with hook(tmpdir, [0]):
        bass2jax.run_bass_via_pjrt(nc, in_maps, n_cores=NCORES)
    ntffs = _glob.glob(_os.path.join(tmpdir, "*_body*.ntff"))
    print(f"profile dir: {tmpdir} ({len(ntffs)} ntff)")
    if not ntffs:
        return None

    import gauge.profiler
    from concourse._compat import FishPath

    prof = gauge.profiler.Profile(
        profile_path=FishPath(tmpdir),
        kernel_dev_mode=True,
        profile_on_exit=False,
        bass_kernel=nc.m,
        offline_processing=True,
        fname="*_body*",
    )
    try:
        res = prof.to_perfetto(model_index=(0,))
        if res:
            print("trace:", res[0].trace_path)
            return res[0].exec_time_ns
    except Exception as e:
        print(f"to_perfetto failed: {type(e).__name__}: {e}")
    return None


# revision 20
# speedup vs baseline: 1.3737x; 1.0469x over previous
"""Trainium2 Bass kernel for nn_CLIPVisionTower_Nuwa_abli (topk_masking).

Per-image pipeline (pure batch data-parallel, 1 image per NeuronCore):
  metric  = sum over heads of attn[:, 0, 1:]                  [576]
  mask out each 2x2-region argmin, take global top-42 of the
  remainder, bench = those patch indices ascending             [42]
  nrm     = l2-normalized mean over 9 layers of hs[:, 1:, :]   [576,1024]
  sim     = nrm[bench] @ nrm.T                                 [42,576]
  w       = relu(sim) * dist_penalty[bench]; row-normalize; self weight 1
  out     = w @ hidden_agg[1:, :]                              [42,1024]
"""

import math

import numpy as np

B = 8
HGRID = 24
P = 576  # patches
K = 42  # bench tokens
HEADS = 16
D = 1024
L = 9  # layers
NCORES = 8
PT = [(0, 128), (128, 128), (256, 128), (384, 128), (512, 64)]  # patch tiles
ND = D // 128  # 8 d-chunks of 128
NEG_BIG = -1.0e30
KEY_BIG = 1.0e9


def _dist_penalty_np() -> np.ndarray:
    ys, xs = np.meshgrid(
        np.arange(HGRID, dtype=np.float32),
        np.arange(HGRID, dtype=np.float32),
        indexing="ij",
    )
    coords = np.stack([ys, xs], axis=-1).reshape(-1, 2)
    diff = coords[:, None, :] - coords[None, :, :]
    dist = np.sqrt((diff * diff).sum(axis=-1), dtype=np.float32)
    thresh = np.float32(math.sqrt(280.0))
    return (np.float32(1.0) - np.minimum(dist / thresh, np.float32(1.0))).astype(
        np.float32
    )


def _emit(nc, tc, ctx):
    import concourse.bass as bass
    import concourse.mybir as mybir
    from concourse.masks import make_identity

    f32 = mybir.dt.float32
    f32r = mybir.dt.float32r
    i32 = mybir.dt.int32
    Alu = mybir.AluOpType
    Act = mybir.ActivationFunctionType
    AX = mybir.AxisListType
    NT = len(PT)

    attn = nc.dram_tensor("attn", [HEADS, P + 1, P + 1], f32, kind="ExternalInput").ap()
    hagg = nc.dram_tensor("hidden_agg", [P + 1, D], f32, kind="ExternalInput").ap()
    hs = nc.dram_tensor("stacked_hs", [L, P + 1, D], f32, kind="ExternalInput").ap()
    dist = nc.dram_tensor("dist", [P, P], f32, kind="ExternalInput").ap()
    out_d = nc.dram_tensor("out", [K, D], f32, kind="ExternalOutput").ap()
    bench_d = nc.dram_tensor("bench", [1, K], i32, kind="ExternalOutput").ap()
    bnrm_dram = nc.dram_tensor("bnrm_scr", [K, D], f32).ap()

    consts = ctx.enter_context(tc.tile_pool(name="consts", bufs=1))
    sel = ctx.enter_context(tc.tile_pool(name="sel", bufs=1))
    persist = ctx.enter_context(tc.tile_pool(name="persist", bufs=1))
    hst_pool = ctx.enter_context(tc.tile_pool(name="hst", bufs=8))
    scr_pool = ctx.enter_context(tc.tile_pool(name="scr", bufs=3))
    ps_t = ctx.enter_context(tc.tile_pool(name="ps_t", bufs=3, space="PSUM"))
    ps_acc = ctx.enter_context(tc.tile_pool(name="ps_acc", bufs=2, space="PSUM"))
    ps_bn = ctx.enter_context(tc.tile_pool(name="ps_bn", bufs=3, space="PSUM"))

    # ---- constants ----
    ident = consts.tile([128, 128], f32, tag="ident")
    make_identity(nc, ident[:])
    ones16 = consts.tile([16, 1], f32, tag="ones16")
    nc.gpsimd.memset(ones16[:], 1.0)
    ones_c = consts.tile([128, 1], f32, tag="ones_c")
    nc.gpsimd.memset(ones_c[:], 1.0)
    # kidx[p, k] = k + 1 (selection-rank match target)
    kidx_i = consts.tile([128, K], i32, tag="kidx_i")
    nc.gpsimd.iota(kidx_i[:], pattern=[[1, K]], base=1, channel_multiplier=0)
    kidx_f = consts.tile([128, K], f32, tag="kidx_f")
    nc.vector.tensor_copy(out=kidx_f[:], in_=kidx_i[:])
    # iota_p[*, p] = p  (patch index along free dim, replicated per partition)
    iota_pi = consts.tile([128, P], i32, tag="iota_pi")
    nc.gpsimd.iota(iota_pi[:], pattern=[[1, P]], base=0, channel_multiplier=0)
    iota_pf = consts.tile([128, P], f32, tag="iota_pf")
    nc.vector.tensor_copy(out=iota_pf[:], in_=iota_pi[:])
    # pgidx[p, t] = 128*t + p (global patch index per tile column)
    pgidx_i = consts.tile([128, NT], i32, tag="pgidx_i")
    nc.gpsimd.iota(pgidx_i[:], pattern=[[128, NT]], base=0, channel_multiplier=1)
    pgidx_f = consts.tile([128, NT], f32, tag="pgidx_f")
    nc.vector.tensor_copy(out=pgidx_f[:], in_=pgidx_i[:])

    # ---- selection: metric = sum_h attn[h, 0, 1:] ----
    cls = sel.tile([HEADS, P], f32, tag="cls")
    nc.sync.dma_start(out=cls[:], in_=attn[:, 0, 1:])
    met_ps1 = ps_acc.tile([1, 512], f32, tag="acc")
    met_ps2 = ps_acc.tile([1, 64], f32, tag="acc")
    nc.tensor.matmul(met_ps1[:], ones16[:], cls[:, 0:512], start=True, stop=True)
    nc.tensor.matmul(met_ps2[:], ones16[:], cls[:, 512:P], start=True, stop=True)
    metric = sel.tile([1, P], f32, tag="metric")
    nc.vector.tensor_copy(out=metric[0:1, 0:512], in_=met_ps1[:])
    nc.vector.tensor_copy(out=metric[0:1, 512:P], in_=met_ps2[:])

    # ---- region (2x2) argmin -> masked metric ----
    # patch p = (2*ry+dy)*24 + 2*rx+dx  ->  [ry(12), dy(2), rx(12), dx(2)]
    met_r = metric[:].rearrange("p (ry dy rx dx) -> p ry dy rx dx", dy=2, rx=12, dx=2)
    rmin = sel.tile([1, 144], f32, tag="rmin")
    rmin2 = sel.tile([1, 144], f32, tag="rmin2")
    rmin_v = rmin[:].rearrange("p (a b) -> p a b", b=12)
    rmin2_v = rmin2[:].rearrange("p (a b) -> p a b", b=12)
    nc.vector.tensor_tensor(
        out=rmin_v, in0=met_r[:, :, 0, :, 0], in1=met_r[:, :, 0, :, 1], op=Alu.min
    )
    nc.vector.tensor_tensor(
        out=rmin2_v, in0=met_r[:, :, 1, :, 0], in1=met_r[:, :, 1, :, 1], op=Alu.min
    )
    nc.vector.tensor_tensor(out=rmin_v, in0=rmin_v, in1=rmin2_v, op=Alu.min)

    masked = sel.tile([1, P], f32, tag="masked")
    msk_r = masked[:].rearrange("p (ry dy rx dx) -> p ry dy rx dx", dy=2, rx=12, dx=2)
    eq = sel.tile([1, 144], f32, tag="eq")
    eq_v = eq[:].rearrange("p (a b) -> p a b", b=12)
    for dy in range(2):
        for dx in range(2):
            src = met_r[:, :, dy, :, dx]
            nc.vector.tensor_tensor(out=eq_v, in0=src, in1=rmin_v, op=Alu.is_equal)
            # masked = metric + eq * NEG_BIG
            nc.vector.scalar_tensor_tensor(
                out=msk_r[:, :, dy, :, dx],
                in0=eq_v,
                scalar=NEG_BIG,
                in1=src,
                op0=Alu.mult,
                op1=Alu.add,
            )

    # ---- global top-42 threshold (6 rounds of max8 / match_replace) ----
    scratch = sel.tile([1, P], f32, tag="scratch")
    nc.vector.tensor_copy(out=scratch[:], in_=masked[:])
    val48 = sel.tile([1, 48], f32, tag="val48")
    for r in range(6):
        nc.vector.max(out=val48[0:1, 8 * r : 8 * (r + 1)], in_=scratch[:])
        nc.vector.match_replace(
            out=scratch[:],
            in_to_replace=val48[0:1, 8 * r : 8 * (r + 1)],
            in_values=scratch[:],
            imm_value=NEG_BIG,
        )
    # mask of selected patches: masked >= v41 (42nd largest)
    selmask = sel.tile([1, P], f32, tag="selmask")
    nc.vector.tensor_scalar(
        out=selmask[:], in0=masked[:], scalar1=val48[0:1, 41:42], scalar2=None,
        op0=Alu.is_ge,
    )
    # rank of each selected patch: pos = inclusive cumsum(selmask); selected
    # patch with rank r has poskey = r+1, unselected 0.
    pos = sel.tile([1, P], f32, tag="pos")
    nc.vector.tensor_tensor_scan(
        out=pos[:], data0=selmask[:], data1=selmask[:], initial=0.0,
        op0=Alu.add, op1=Alu.bypass,
    )
    poskey = sel.tile([1, P], f32, tag="poskey")
    nc.vector.tensor_tensor(out=poskey[:], in0=pos[:], in1=selmask[:], op=Alu.mult)

    # redistribute poskey to partitions (PE transpose per patch tile), then
    # one-hot O[p, k] = (poskey[p] == k+1)
    o_tiles = []
    pscat_tiles = []
    for ti, (p0, rows) in enumerate(PT):
        psP = ps_bn.tile([128, 1], f32, tag="ps_bn")
        nc.tensor.transpose(
            psP[0:rows, 0:1], poskey[0:1, p0 : p0 + rows], ident[0:1, 0:1]
        )
        pcol = sel.tile([rows, 1], f32, tag=f"pcol{ti}", name=f"pcol{ti}")
        nc.scalar.copy(out=pcol[:], in_=psP[0:rows, 0:1])
        # scatter slot: poskey-1 for selected rows, 1000 (dropped) otherwise
        pm1 = sel.tile([rows, 1], f32, tag=f"pm1{ti}", name=f"pm1{ti}")
        nc.vector.tensor_scalar(
            out=pm1[:], in0=pcol[:], scalar1=1.0, scalar2=None, op0=Alu.subtract
        )
        unsel = sel.tile([rows, 1], f32, tag=f"unsel{ti}", name=f"unsel{ti}")
        nc.vector.tensor_scalar(
            out=unsel[:], in0=pcol[:], scalar1=0.5, scalar2=None, op0=Alu.is_lt
        )
        pscat = sel.tile([rows, 1], i32, tag=f"pscat{ti}", name=f"pscat{ti}")
        nc.vector.scalar_tensor_tensor(
            out=pscat[:], in0=unsel[:], scalar=1001.0, in1=pm1[:],
            op0=Alu.mult, op1=Alu.add,
        )
        pscat_tiles.append(pscat)
        ot = sel.tile([rows, K], f32, tag=f"oh{ti}", name=f"oh{ti}")
        nc.vector.tensor_scalar(
            out=ot[:], in0=kidx_f[0:rows, :], scalar1=pcol[:], scalar2=None,
            op0=Alu.is_equal,
        )
        o_tiles.append(ot)

    # bench values: bench[k] = sum_p p * O[p, k]  (exact small ints in f32)
    psV = ps_bn.tile([1, K], f32, tag="ps_bn")
    for ti, (p0, rows) in enumerate(PT):
        nc.tensor.matmul(
            psV[:], pgidx_f[0:rows, ti : ti + 1], o_tiles[ti][:],
            start=(ti == 0), stop=(ti == NT - 1),
        )
    bench_i = sel.tile([1, K], i32, tag="bench_i")
    nc.vector.tensor_copy(out=bench_i[:], in_=psV[:])
    nc.sync.dma_start(out=bench_d, in_=bench_i[:])

    # bench as an int32 column (one index per partition) for indirect gathers
    benchfr = sel.tile([1, K], f32, tag="benchfr")
    nc.scalar.copy(out=benchfr[:], in_=psV[:])
    psC = ps_bn.tile([128, 1], f32, tag="ps_bn")
    nc.tensor.transpose(psC[0:K, 0:1], benchfr[:], ident[0:1, 0:1])
    benchcol = sel.tile([K, 1], i32, tag="benchcol")
    nc.vector.tensor_copy(out=benchcol[:], in_=psC[0:K, 0:1])

    # dpen[k, :] = dist_penalty[bench[k], :] via indirect row gather
    dpen = sel.tile([K, P], f32, tag="dpen")
    nc.gpsimd.indirect_dma_start(
        out=dpen[:],
        out_offset=None,
        in_=dist,
        in_offset=bass.IndirectOffsetOnAxis(ap=benchcol[:, 0:1], axis=0),
    )

    # oself[k, p] = (p == bench[k]) int mask for copy_predicated
    benchcf = sel.tile([K, 1], f32, tag="benchcf")
    nc.scalar.copy(out=benchcf[:], in_=psC[0:K, 0:1])
    oself = sel.tile([K, P], i32, tag="oself")
    nc.vector.tensor_scalar(
        out=oself[:], in0=iota_pf[0:K, :], scalar1=benchcf[:], scalar2=None,
        op0=Alu.is_equal,
    )

    # ---- heavy phase: layer sum (scale dropped; nrm is scale-invariant) ----
    nrm_tiles = []
    nrmT = [
        persist.tile([128, P], f32r, tag=f"nrmT{dt}", name=f"nrmT{dt}")
        for dt in range(ND)
    ]
    LA, LB = 4, 5
    for ti, (p0, rows) in enumerate(PT):
        # stream 9 layers into per-layer slots (freed right after their add);
        # DMA issue split across the two HWDGE queues (sync + scalar)
        lt = []
        for l in range(L):
            t = hst_pool.tile([rows, D], f32, tag="hst", name="hst")
            dmae = nc.sync if l % 2 == 0 else nc.scalar
            dmae.dma_start(out=t[:], in_=hs[l, 1 + p0 : 1 + p0 + rows, :])
            lt.append(t)
        accV = scr_pool.tile([rows, D], f32, tag="accV")
        nc.vector.tensor_add(accV[:], lt[0][:], lt[1][:])
        nc.vector.tensor_add(accV[:], accV[:], lt[2][:])
        nc.vector.tensor_add(accV[:], accV[:], lt[3][:])
        nc.vector.tensor_add(accV[:], accV[:], lt[4][:])
        nc.vector.tensor_add(accV[:], accV[:], lt[5][:])
        accG = scr_pool.tile([rows, D], f32, tag="accG")
        nc.gpsimd.tensor_add(accG[:], lt[6][:], lt[7][:])
        nc.gpsimd.tensor_add(accG[:], accG[:], lt[8][:])
        ssum = scr_pool.tile([rows, D], f32, tag="ssum")
        nc.vector.tensor_add(ssum[:], accV[:], accG[:])

        # l2 normalize rows
        sumsq = scr_pool.tile([rows, 1], f32, tag="sumsq")
        sq = scr_pool.tile([rows, D], f32, tag="sq", bufs=1)
        nc.scalar.activation(
            out=sq[:], in_=ssum[:], func=Act.Square, accum_out=sumsq[:]
        )
        nrmv = scr_pool.tile([rows, 1], f32, tag="nrmv")
        nc.scalar.activation(out=nrmv[:], in_=sumsq[:], func=Act.Sqrt)
        nc.vector.tensor_scalar_max(nrmv[:], nrmv[:], 1e-12)
        inv = scr_pool.tile([rows, 1], f32, tag="inv")
        nc.vector.reciprocal(out=inv[:], in_=nrmv[:])
        nrmt = persist.tile([rows, D], f32, tag=f"nrm{ti}", name=f"nrm{ti}")
        nc.vector.tensor_scalar(
            out=nrmt[:], in0=ssum[:], scalar1=inv[:], scalar2=None, op0=Alu.mult
        )
        nrm_tiles.append(nrmt)

        # scatter this block's selected rows into the bench-row scratch
        nc.gpsimd.indirect_dma_start(
            out=bnrm_dram,
            out_offset=bass.IndirectOffsetOnAxis(
                ap=pscat_tiles[ti][:, 0:1], axis=0
            ),
            in_=nrmt[:],
            in_offset=None,
            bounds_check=K - 1,
            oob_is_err=False,
        )

        # transpose into nrmT d-chunks
        for dt in range(ND):
            psT = ps_t.tile([128, rows], f32, tag="ps_t")
            nc.tensor.transpose(
                psT[:], nrmt[:, 128 * dt : 128 * (dt + 1)], ident[0:rows, 0:rows]
            )
            nc.scalar.copy(out=nrmT[dt][:, p0 : p0 + rows], in_=psT[:])

    # agg tiles last: their DMA overlaps the PE tail
    agg_tiles = []
    for ti, (p0, rows) in enumerate(PT):
        aggt = persist.tile([rows, D], f32r, tag=f"agg{ti}", name=f"agg{ti}")
        nc.gpsimd.dma_start(out=aggt[:], in_=hagg[1 + p0 : 1 + p0 + rows, :])
        agg_tiles.append(aggt)

    # ---- bench_nrm rows (scattered above), transpose to d-major ----
    bnrm = sel.tile([K, D], f32, tag="bnrm")
    nc.sync.dma_start(out=bnrm[:], in_=bnrm_dram)
    bnrmT = []
    for dt in range(ND):
        psBT = ps_bn.tile([128, K], f32, tag="ps_bn")
        nc.tensor.transpose(
            psBT[:], bnrm[:, 128 * dt : 128 * (dt + 1)], ident[0:K, 0:K]
        )
        bt = sel.tile([128, K], f32r, tag=f"bnrmT{dt}", name=f"bnrmT{dt}")
        nc.scalar.copy(out=bt[:], in_=psBT[:])
        bnrmT.append(bt)

    # ---- sim = bench_nrm @ nrm^T  [42, 576] ----
    sim1 = ps_acc.tile([K, 512], f32, tag="acc")
    sim2 = ps_acc.tile([K, 64], f32, tag="acc")
    for dt in range(ND):
        nc.tensor.matmul(
            sim1[:], bnrmT[dt][:], nrmT[dt][:, 0:512],
            start=(dt == 0), stop=(dt == ND - 1),
        )
        nc.tensor.matmul(
            sim2[:], bnrmT[dt][:], nrmT[dt][:, 512:P],
            start=(dt == 0), stop=(dt == ND - 1),
        )

    # ---- w = relu(sim) * dpen; normalize; self weight 1.0 ----
    w = sel.tile([K, P], f32, tag="w")
    nc.scalar.activation(out=w[:, 0:512], in_=sim1[:], func=Act.Relu)
    nc.scalar.activation(out=w[:, 512:P], in_=sim2[:], func=Act.Relu)
    nc.vector.tensor_tensor(out=w[:], in0=w[:], in1=dpen[:], op=Alu.mult)
    wsum = sel.tile([K, 1], f32, tag="wsum")
    nc.vector.tensor_reduce(out=wsum[:], in_=w[:], axis=AX.X, op=Alu.add)
    nc.vector.tensor_scalar_add(wsum[:], wsum[:], 1e-8)
    winv = sel.tile([K, 1], f32, tag="winv")
    nc.vector.reciprocal(out=winv[:], in_=wsum[:])
    nc.vector.tensor_scalar(
        out=w[:], in0=w[:], scalar1=winv[:], scalar2=None, op0=Alu.mult
    )
    nc.vector.copy_predicated(
        out=w[:], mask=oself[:], data=ones_c[0:K, 0:1].to_broadcast([K, P])
    )

    # ---- out = w @ patch_agg  [42, 1024] ----
    wT = []
    for ti, (p0, rows) in enumerate(PT):
        psW = ps_t.tile([128, K], f32, tag="ps_t")
        nc.tensor.transpose(psW[0:rows, 0:K], w[:, p0 : p0 + rows], ident[0:K, 0:K])
        wt = sel.tile([rows, K], f32r, tag=f"wT{ti}", name=f"wT{ti}")
        nc.scalar.copy(out=wt[:], in_=psW[0:rows, 0:K])
        wT.append(wt)
    o1 = ps_acc.tile([K, 512], f32, tag="acc")
    o2 = ps_acc.tile([K, 512], f32, tag="acc")
    for ti, (p0, rows) in enumerate(PT):
        nc.tensor.matmul(
            o1[:], wT[ti][:], agg_tiles[ti][:, 0:512],
            start=(ti == 0), stop=(ti == NT - 1),
        )
        nc.tensor.matmul(
            o2[:], wT[ti][:], agg_tiles[ti][:, 512:D],
            start=(ti == 0), stop=(ti == NT - 1),
        )
    outsb = sel.tile([K, D], f32, tag="outsb")
    nc.vector.tensor_copy(out=outsb[:, 0:512], in_=o1[:])
    nc.scalar.copy(out=outsb[:, 512:D], in_=o2[:])
    nc.sync.dma_start(out=out_d, in_=outsb[:])


def build():
    from contextlib import ExitStack

    import concourse.bacc as bacc
    from concourse.tile import TileContext

    nc = bacc.Bacc("TRN2")
    with TileContext(nc) as tc:
        with ExitStack() as ctx:
            _emit(nc, tc, ctx)
    nc.compile()
    return nc


_NC_CACHE = {}


def kernel(attn, hidden_agg, stacked_hs):
    import numpy as np

    from concourse.bass_utils import run_bass_kernel_spmd

    if "nc" not in _NC_CACHE:
        _NC_CACHE["nc"] = build()
    nc = _NC_CACHE["nc"]

    dist = _dist_penalty_np()
    in_maps = [
        {
            "attn": np.ascontiguousarray(attn[b]),
            "hidden_agg": np.ascontiguousarray(hidden_agg[b]),
            "stacked_hs": np.ascontiguousarray(stacked_hs[:, b]),
            "dist": dist,
        }
        for b in range(NCORES)
    ]
    res = run_bass_kernel_spmd(nc, in_maps, list(range(NCORES)))
    out = np.stack([res.results[b]["out"] for b in range(NCORES)]).astype(np.float32)
    bench = np.stack(
        [res.results[b]["bench"].reshape(K) for b in range(NCORES)]
    ).astype(np.int32)
    return out, bench


def profile(inputs, tmpdir=None):
    """Run once under NTFF capture; returns HW exec time in ns (or None).

    Leaves the ntff/pftrace artifacts in ``tmpdir`` for trace analysis.
    """
    import glob as _glob
    import os as _os
    import tempfile

    import numpy as np

    from concourse import bass2jax

    try:
        from trn_agent_boot.trn_boot import _ntff_profile_via_ctypes
    except ImportError:
        return None
    hook = _ntff_profile_via_ctypes("/opt/axon/libaxon_pjrt.so")
    if hook is None:
        return None

    if "nc" not in _NC_CACHE:
        _NC_CACHE["nc"] = build()
    nc = _NC_CACHE["nc"]
    dist = _dist_penalty_np()
    in_maps = [
        {
            "attn": np.ascontiguousarray(inputs["attn"][b]),
            "hidden_agg": np.ascontiguousarray(inputs["hidden_agg"][b]),
            "stacked_hs": np.ascontiguousarray(inputs["stacked_hs"][:, b]),
            "dist": dist,
        }
        for b in range(NCORES)
    ]
    tmpdir = tmpdir or tempfile.mkdtemp(prefix="ntffprof_")
    with hook(tmpdir, [0]):
        bass2jax.run_bass_via_pjrt(nc, in_maps, n_cores=NCORES)
    ntffs = _glob.glob(_os.path.join(tmpdir, "*_body*.ntff"))
    print(f"profile dir: {tmpdir} ({len(ntffs)} ntff)")
    if not ntffs:
        return None

    import gauge.profiler
    from concourse._compat import FishPath

    prof = gauge.profiler.Profile(
        profile_path=FishPath(tmpdir),
        kernel_dev_mode=True,
        profile_on_exit=False,
        bass_kernel=nc.m,
        offline_processing=True,
        fname="*_body*",
    )
    try:
        res = prof.to_perfetto(model_index=(0,))
        if res:
            print("trace:", res[0].trace_path)
            return res[0].exec_time_ns
    except Exception as e:
        print(f"to_perfetto failed: {type(e).__name__}: {e}")
    return None
